# revision 36
# baseline (speedup 1.0000x reference)
"""GPT (4-layer, E=768, H=12, T=1024, B=2, V=50257) forward on 8 trn2 cores.

Sharding:
  - Residual stream x token-sharded fp32: core c owns tokens [c*256,(c+1)*256)
    of the flattened [2048] (batch-major): cores 0-3 = batch 0, 4-7 = batch 1.
  - Attention head-sharded within each batch group of 4 cores (3 heads each):
    AllGather hidden (fp16, split in 2 halves for overlap), compute q/k/v +
    scores + y for my heads over all 1024 tokens, then AllToAll the normalized
    per-head y back to token owners (uniform SPMD), out-proj token-local with
    full 768 contraction (fp32 psum, no low-precision reduction anywhere).
  - MLP fully token-local: fc weight-stationary (out [hid,tok]),
    pr activation-stationary (out [tok,E]) - no transposes inside MLP.
  - lm_head vocab-sharded fp16: AllGather lnf(x) (all 8), each core computes
    [2048, 6284] logit slice, vocab-group-outer loop with resident xfT.
  - All matmul inputs fp16 (fp32 psum accumulate); scores 2-head row-packed
    (tile_position concurrency); h2 on row-group 64-127.
"""

import sys
from contextlib import ExitStack
import numpy as np

sys.path.insert(0, "/opt/trn_rl_repo")

import concourse.bass as bass
import concourse.mybir as mybir
import concourse.tile as tile
from concourse import bacc
from concourse.bass_utils import run_bass_kernel_spmd
from concourse.masks import make_identity

L, H, E, T, V = 4, 12, 768, 1024, 50257
B = 2
NC = 8
TS = (B * T) // NC          # 256 tokens per core
VS = 6284                   # vocab slice per core (padded V = 50272)
VPAD = VS * NC
HD = 64
EPS = 1e-5
SCALE = float(1.0 / np.sqrt(np.float32(E)))
F32 = mybir.dt.float32
F16 = mybir.dt.float16

L_RUN = L  # layers actually executed (tests may truncate)
SIM_GELU = False  # sim lacks Gelu_apprx_tanh; use x*sigmoid(1.702x) for debug
DEBUG_DUMP = False  # dump layer-0 intermediates to a "dbg" output
_CACHE = {}


def _build_program():
    nc = bacc.Bacc("TRN2", target_bir_lowering=False, debug=False, num_devices=NC)

    # ---- I/O -------------------------------------------------------------
    x0s = nc.dram_tensor("x0s", [TS, E], F32, kind="ExternalInput")
    # wqk cols: [q0|q1 (128), k0|k1 (128), pad|q2 (128), pad|k2 (128)]
    wqk = nc.dram_tensor("wqk", [L, E, 512], F16, kind="ExternalInput")
    bqk = nc.dram_tensor("bqk", [L, 128, 4], F32, kind="ExternalInput")
    wv = nc.dram_tensor("wv", [L, E, 3 * HD], F16, kind="ExternalInput")
    watp = nc.dram_tensor("watp", [L, E, E], F16, kind="ExternalInput")  # full (head-major rows)
    atpb = nc.dram_tensor("atpb", [L, E], F32, kind="ExternalInput")     # includes bv@watp fold
    fcw = nc.dram_tensor("fcw", [L, E, 4 * E], F16, kind="ExternalInput")
    fcb = nc.dram_tensor("fcb", [L, 128, 24], F32, kind="ExternalInput")
    prw = nc.dram_tensor("prw", [L, 4 * E, E], F16, kind="ExternalInput")
    prb = nc.dram_tensor("prb", [L, E], F32, kind="ExternalInput")
    ln1g = nc.dram_tensor("ln1g", [L, E], F32, kind="ExternalInput")
    ln1b = nc.dram_tensor("ln1b", [L, E], F32, kind="ExternalInput")
    ln2g = nc.dram_tensor("ln2g", [L, E], F32, kind="ExternalInput")
    ln2b = nc.dram_tensor("ln2b", [L, E], F32, kind="ExternalInput")
    lnfg = nc.dram_tensor("lnfg", [1, E], F32, kind="ExternalInput")
    lnfb = nc.dram_tensor("lnfb", [1, E], F32, kind="ExternalInput")
    wteT = nc.dram_tensor("wteT", [(VS + 511) // 512, 6, 128, 512], F16, kind="ExternalInput")
    NCH = (VS + 511) // 512
    logits = nc.dram_tensor("logits", [NCH * 16 * 128, 512], F16, kind="ExternalOutput")
    dbg = (nc.dram_tensor("dbg", [12, 128, T], F16, kind="ExternalOutput")
           if DEBUG_DUMP else None)

    tri_np = (np.arange(128)[None, :] >= np.arange(128)[:, None]).astype(np.float16)
    tri = nc.inline_tensor(tri_np, name="tri_const")

    g_all = [list(range(NC))]
    g_batch = [[0, 1, 2, 3], [4, 5, 6, 7]]

    def bcast_row(pool, src_ap, n, dtype, w, tag=None):
        """Replicate a [w] DRAM row across n partitions via broadcast DMA."""
        t = pool.tile([n, w], dtype, tag=tag)
        in_ap = bass.AP(
            tensor=src_ap.tensor,
            offset=src_ap.offset,
            ap=[[0, n]] + [list(p) for p in src_ap.ap],
        )
        eng = nc.gpsimd if dtype != src_ap.dtype else nc.sync
        eng.dma_start(out=t[:], in_=in_ap)
        return t

    with tile.TileContext(nc) as tc, ExitStack() as es:
        const = es.enter_context(tc.tile_pool(name="const", bufs=1))
        xp = es.enter_context(tc.tile_pool(name="xp", bufs=1))
        lnrow = es.enter_context(tc.tile_pool(name="lnrow", bufs=1))
        stat = es.enter_context(tc.tile_pool(name="stat", bufs=2))
        hpool = es.enter_context(tc.tile_pool(name="hpool", bufs=2))
        dram = es.enter_context(tc.tile_pool(name="dram", bufs=1, space="DRAM"))

        ident_f = const.tile([128, 128], F32, name="ident_f")
        make_identity(nc, ident_f)
        ident = const.tile([128, 128], F16, name="ident")
        nc.vector.tensor_copy(ident[:], ident_f[:])
        tri_sb = const.tile([128, 128], F16, name="tri_sb")
        nc.sync.dma_start(out=tri_sb[:], in_=tri[:, :])
        eps_sb = const.tile([128, 1], F32, name="eps_sb")
        nc.vector.memset(eps_sb, EPS)
        ones3 = const.tile([128, 3], F16, name="ones3")
        nc.vector.memset(ones3, 1.0)

        # persistent residual stream [256, 768] fp32 as two [128, 768] tiles
        x_sb = [xp.tile([128, E], F32, tag=f"x{t}", name=f"x{t}") for t in range(2)]
        for t in range(2):
            nc.sync.dma_start(out=x_sb[t][:], in_=x0s[t * 128:(t + 1) * 128, :])

        # DRAM bounce buffers for collectives (fp16)
        hT_in = [dram.tile([3 * 128, TS], F16, name=f"hT_in{i}") for i in range(2)]
        hT_ag = [dram.tile([4 * 3 * 128, TS], F16, name=f"hT_ag{i}") for i in range(2)]
        y_in_y = dram.tile([3 * HD, T], F16, name="y_in_y")
        y_ag = dram.tile([4 * 3 * HD, T], F16, name="y_ag")
        xf_in = [dram.tile([3 * 128, TS], F16, name=f"xf_in{i}") for i in range(2)]
        xf_ag = [dram.tile([NC * 3 * 128, TS], F16, name=f"xf_ag{i}") for i in range(2)]

        def layernorm_t(x_ap, g_bc, b_bc, out_tile):
            """LN over free dim (768) of [128, 768] fp32 tile -> out fp16."""
            stats = stat.tile([128, 3, 6], F32, tag="bn_stats", name="bn_stats_t")
            xr = x_ap.rearrange("p (s d) -> p s d", s=3)
            for s in range(3):
                nc.vector.bn_stats(out=stats[:, s, :], in_=xr[:, s, :])
            mv = stat.tile([128, 2], F32, tag="bn_aggr", name="bn_aggr_t")
            nc.vector.bn_aggr(out=mv[:], in_=stats[:])
            rstd = stat.tile([128, 1], F32, tag="rstd", name="rstd_t")
            nc.scalar.activation(out=rstd[:], in_=mv[:, 1:2],
                                 func=mybir.ActivationFunctionType.Sqrt,
                                 bias=eps_sb[:], scale=1.0)
            nc.vector.reciprocal(out=rstd[:], in_=rstd[:])
            tmp = stat.tile([128, E], F32, tag="ln_tmp", name="ln_tmp")
            nc.vector.tensor_scalar(out=tmp[:], in0=x_ap,
                                    scalar1=mv[:, 0:1], scalar2=rstd[:],
                                    op0=mybir.AluOpType.subtract,
                                    op1=mybir.AluOpType.mult)
            nc.vector.tensor_mul(out=tmp[:], in0=tmp[:], in1=g_bc[:])
            nc.vector.tensor_add(out=out_tile[:], in0=tmp[:], in1=b_bc[:])

        # ---- persistent layer pools (tags reused across layers) ----------
        es_l = es.enter_context(ExitStack())
        wqkp = es_l.enter_context(tc.tile_pool(name="wqkp", bufs=1))
        wvp = es_l.enter_context(tc.tile_pool(name="wvp", bufs=1))
        watpp = es_l.enter_context(tc.tile_pool(name="watpp", bufs=1))
        fcwp = es_l.enter_context(tc.tile_pool(name="fcwp", bufs=1))
        prwp = es_l.enter_context(tc.tile_pool(name="prwp", bufs=1))
        bias_p = es_l.enter_context(tc.tile_pool(name="bias_p", bufs=2))
        hTp = es_l.enter_context(tc.tile_pool(name="hTp", bufs=1))
        hTbp = es_l.enter_context(tc.tile_pool(name="hTbp", bufs=1))
        qkp = es_l.enter_context(tc.tile_pool(name="qkp", bufs=1))
        vp = es_l.enter_context(tc.tile_pool(name="vp", bufs=1))
        ep = es_l.enter_context(tc.tile_pool(name="ep", bufs=2))
        yp = es_l.enter_context(tc.tile_pool(name="yp", bufs=1))
        sm = es_l.enter_context(tc.tile_pool(name="sm", bufs=2))
        mTp = es_l.enter_context(tc.tile_pool(name="mTp", bufs=1))
        yallp = es_l.enter_context(tc.tile_pool(name="yallp", bufs=1))

        # v_sb layout [128, 258]: h0 [v0|1] at 0:65, h1 [1|pad63|v1] at
        # 65:193 (den at psum row 0, y1 at rows 64-127 for packed
        # out-proj), h2 [v2|1] at 193:258. Constant cols written once.
        v_sb = [vp.tile([128, 258], F16, tag=f"v{t}", name=f"v{t}") for t in range(8)]
        for t in range(8):
            nc.vector.tensor_copy(v_sb[t][:, 64:65], ones3[:, 0:1])
            nc.vector.tensor_copy(v_sb[t][:, 65:66], ones3[:, 1:2])
            nc.vector.tensor_copy(v_sb[t][:, 257:258], ones3[:, 2:3])
            nc.vector.memset(v_sb[t][:, 66:129], 0.0)

        for layer in range(L_RUN):
            # ---- weight loads (gpsimd queue; Tile schedules early) ------
            wqk_sb = [wqkp.tile([128, 512], F16, tag=f"wqk{k}", name=f"wqk{k}") for k in range(6)]
            wv_sb = [wvp.tile([128, 192], F16, tag=f"wv{k}", name=f"wv{k}") for k in range(6)]
            watp_sb = [watpp.tile([128, E], F16, tag=f"wa{k}", name=f"wa{k}") for k in range(6)]
            for k in range(6):
                nc.scalar.dma_start(out=watp_sb[k][:], in_=watp[layer, k * 128:(k + 1) * 128, :])
                nc.sync.dma_start(out=wqk_sb[k][:], in_=wqk[layer, k * 128:(k + 1) * 128, :])
                nc.sync.dma_start(out=wv_sb[k][:], in_=wv[layer, k * 128:(k + 1) * 128, :])
            fcw_sb = [fcwp.tile([128, 4 * E], F16, tag=f"fcw{k}", name=f"fcw{k}") for k in range(6)]
            for k in range(6):
                nc.scalar.dma_start(out=fcw_sb[k][:], in_=fcw[layer, k * 128:(k + 1) * 128, :])
            prw_sb = [prwp.tile([128, E], F16, tag=f"prw{m}", name=f"prw{m}") for m in range(24)]
            for m in range(24):
                (nc.scalar if m % 2 else nc.sync).dma_start(
                    out=prw_sb[m][:], in_=prw[layer, m * 128:(m + 1) * 128, :])
            bqk_sb = bias_p.tile([128, 4], F32, tag="bqk", name="bqk")
            nc.sync.dma_start(out=bqk_sb[:], in_=bqk[layer])
            fcb_sb = bias_p.tile([128, 24], F32, tag="fcb", name="fcb")
            nc.sync.dma_start(out=fcb_sb[:], in_=fcb[layer])
            ln1g_bc = bcast_row(lnrow, ln1g[layer], 128, F16, E, tag="ln1g")
            ln1b_bc = bcast_row(lnrow, ln1b[layer], 128, F16, E, tag="ln1b")

            # ---- LN1 + transpose -> hT [768, 256] fp16 -------------------
            es_t1 = ExitStack()
            psT = es_t1.enter_context(tc.tile_pool(name="psT", bufs=4, space="PSUM"))
            hT = [hTp.tile([128, TS], F16, tag=f"hT{k}", name=f"hT{k}") for k in range(6)]
            for t in range(2):
                h_t = hpool.tile([128, E], F16, tag="h", name="h")
                layernorm_t(x_sb[t][:], ln1g_bc, ln1b_bc, h_t)
                for k in range(6):
                    pt = psT.tile([128, 128], F16, tag="tr", name="tr")
                    nc.tensor.transpose(pt[:], h_t[:, k * 128:(k + 1) * 128], ident[:])
                    dst = hT[k][:, t * 128:(t + 1) * 128]
                    if k % 2 == 0:
                        nc.vector.tensor_copy(dst, pt[:])
                    else:
                        nc.scalar.activation(out=dst, in_=pt[:],
                                             func=mybir.ActivationFunctionType.Copy)
            es_t1.close()

            # ---- AllGather hidden in 2 halves (within batch group of 4) --
            for half in range(2):
                for k in range(3):
                    nc.sync.dma_start(out=hT_in[half][k * 128:(k + 1) * 128, :],
                                      in_=hT[half * 3 + k][:])
                nc.gpsimd.collective_compute(
                    "AllGather", mybir.AluOpType.bypass,
                    replica_groups=g_batch,
                    ins=[hT_in[half].opt()],
                    outs=[hT_ag[half].opt()],
                )
            # load hTb 6 x [128, 1024] fp16 (one 3D-AP DMA per k-chunk)
            hTb = [hTbp.tile([128, T], F16, tag=f"hTb{k}", name=f"hTb{k}") for k in range(6)]
            for k in range(6):
                half, kk = divmod(k, 3)
                src = hT_ag[half]
                in_ap = bass.AP(
                    tensor=src.tensor,
                    offset=src[kk * 128, 0].offset,
                    ap=[[TS, 128], [3 * 128 * TS, 4], [1, TS]],
                )
                eng = (nc.sync, nc.scalar)[k % 2]
                eng.dma_start(
                    out=hTb[k][:].rearrange("p (r t) -> p r t", r=4),
                    in_=in_ap)

            if DEBUG_DUMP and layer == 0:
                nc.sync.dma_start(out=dbg[0], in_=hTb[0][:])
                nc.sync.dma_start(out=dbg[11][:, 0:128], in_=tri_sb[:])

            # ---- QKV ----------------------------------------------------
            es_a = ExitStack()
            psQK = es_a.enter_context(tc.tile_pool(name="psQK", bufs=3, space="PSUM"))
            psV = es_a.enter_context(tc.tile_pool(name="psV", bufs=2, space="PSUM"))
            # qT2/kT2: rows 0-63 head0, 64-127 head1; q1T/k1T rows 64-127 head2
            qT2 = qkp.tile([128, T], F16, tag="qT2", name="qT2")
            kT2 = qkp.tile([128, T], F16, tag="kT2", name="kT2")
            q1T = qkp.tile([128, T], F16, tag="q1T", name="q1T")
            k1T = qkp.tile([128, T], F16, tag="k1T", name="k1T")
            qk_dsts = [qT2, kT2, q1T, k1T]
            for s in range(4):
                ps = psQK.tile([128, T], F32, tag="qk", name="qk")
                for n in range(2):
                    for k in range(6):
                        nc.tensor.matmul(ps[:, n * 512:(n + 1) * 512],
                                         wqk_sb[k][:, s * 128:(s + 1) * 128],
                                         hTb[k][:, n * 512:(n + 1) * 512],
                                         start=(k == 0), stop=(k == 5))
                if s < 2:
                    nc.vector.tensor_scalar_add(out=qk_dsts[s][:], in0=ps[:],
                                                scalar1=bqk_sb[:, s:s + 1])
                else:
                    nc.vector.tensor_scalar_add(out=qk_dsts[s][64:128, :],
                                                in0=ps[64:128, :],
                                                scalar1=bqk_sb[64:128, s:s + 1])
            for t in range(8):
                ps = psV.tile([128, 192], F32, tag="v", name="v")
                for k in range(6):
                    nc.tensor.matmul(ps[:], hTb[k][:, t * 128:(t + 1) * 128],
                                     wv_sb[k][:], start=(k == 0), stop=(k == 5))
                if t % 2 == 0:
                    nc.vector.tensor_copy(v_sb[t][:, 0:64], ps[:, 0:64])
                    nc.vector.tensor_copy(v_sb[t][:, 129:193], ps[:, 64:128])
                    nc.vector.tensor_copy(v_sb[t][:, 193:257], ps[:, 128:192])
                else:
                    nc.scalar.activation(out=v_sb[t][:, 0:64], in_=ps[:, 0:64],
                                         func=mybir.ActivationFunctionType.Copy)
                    nc.scalar.activation(out=v_sb[t][:, 129:193], in_=ps[:, 64:128],
                                         func=mybir.ActivationFunctionType.Copy)
                    nc.scalar.activation(out=v_sb[t][:, 193:257], in_=ps[:, 128:192],
                                         func=mybir.ActivationFunctionType.Copy)

            if DEBUG_DUMP and layer == 0:
                nc.sync.dma_start(out=dbg[1], in_=qT2[:])
                nc.sync.dma_start(out=dbg[2], in_=kT2[:])
                nc.sync.dma_start(out=dbg[3], in_=k1T[:])
                nc.sync.dma_start(out=dbg[10][:, 0:258], in_=v_sb[0][:])

            # ---- attention: scores + y, head 0/1 row-packed --------------
            es_a.close()
            es_b = ExitStack()
            psS = es_b.enter_context(tc.tile_pool(name="psS", bufs=2, space="PSUM"))
            psY = es_b.enter_context(tc.tile_pool(name="psY", bufs=1, space="PSUM"))
            # h0: y rows 0:64, den 64; h1: den 0, y 64:128; h2: y 0:64, den 64
            yps0 = psY.tile([65, T], F32, tag="y0", name="y0")
            yps1 = psY.tile([128, T], F32, tag="y1", name="y1")
            yps2 = psY.tile([65, T], F32, tag="y2", name="y2")
            heads = [(qT2[0:64, :], kT2[0:64, :], yps0, 0, 65),
                     (qT2[64:128, :], kT2[64:128, :], yps1, 65, 193),
                     (q1T[64:128, :], k1T[64:128, :], yps2, 193, 258)]
            for j in range(8):
                qs = j * 128
                qlen = T - qs
                for h in range(3):
                    qT_h, kT_h, yout, v0, v1 = heads[h]
                    e_sb = ep.tile([128, T], F16, tag=f"e{h}", name=f"e{h}")
                    off = 0
                    while off < qlen:
                        cl = min(512, qlen - off)
                        pss = psS.tile([128, 512], F32, tag="s", name="s")
                        nc.tensor.matmul(pss[:, 0:cl], kT_h[:, qs:qs + 128],
                                         qT_h[:, qs + off: qs + off + cl],
                                         start=True, stop=True)
                        nc.scalar.activation(out=e_sb[:, off:off + cl],
                                             in_=pss[:, 0:cl],
                                             func=mybir.ActivationFunctionType.Exp,
                                             scale=SCALE)
                        off += cl
                    nc.vector.tensor_mul(out=e_sb[:, 0:128], in0=e_sb[:, 0:128],
                                         in1=tri_sb[:])
                    if qs < 512:
                        nc.tensor.matmul(yout[:, qs:512],
                                         v_sb[j][:, v0:v1],
                                         e_sb[:, 0:512 - qs],
                                         start=(j == 0), stop=(j == 3))
                    nc.tensor.matmul(yout[:, max(qs, 512):T],
                                     v_sb[j][:, v0:v1],
                                     e_sb[:, max(qs, 512) - qs:qlen],
                                     start=(j == 0), stop=(j == 7))

            # ---- normalize into packed y tiles ---------------------------
            # y01T rows 0:64 = head0, 64:128 = head1; y2T rows 0:64 = head2
            y01T = yp.tile([128, T], F16, tag="y01T", name="y01T")
            y2T = yp.tile([64, T], F16, tag="y2T", name="y2T")
            for h, (yout, den_row, dst) in enumerate(
                    [(yps0, 64, y01T[0:64, :]), (yps1, 0, y01T[64:128, :]),
                     (yps2, 64, y2T[:])]):
                # den psum->sbuf (gpsimd cannot read PSUM), broadcast to all
                # 128 partitions, then one DVE divide
                den_sb = sm.tile([1, T], F32, tag="den", name="den")
                nc.vector.tensor_copy(den_sb[:], yout[den_row:den_row + 1, :])
                bcf = sm.tile([128, T], F32, tag="bcf", name="bcf")
                nc.gpsimd.partition_broadcast(bcf[:], den_sb[:])
                lo = 64 if h == 1 else 0
                ysrc = yout[64:128, :] if h == 1 else yout[0:64, :]
                nc.vector.reciprocal_approx_fast(out=bcf[:], in_=bcf[:])
                nc.vector.tensor_mul(out=dst, in0=ysrc, in1=bcf[lo:lo + 64, :])
            if DEBUG_DUMP and layer == 0:
                nc.sync.dma_start(out=dbg[4], in_=y01T[:])
                nc.sync.dma_start(out=dbg[5][0:64, :], in_=y2T[:])
            es_b.close()

            # ---- AllGather y (all heads, all tokens) ---------------------
            nc.sync.dma_start(out=y_in_y[0:128, :], in_=y01T[:])
            nc.sync.dma_start(out=y_in_y[128:192, :], in_=y2T[:])
            nc.gpsimd.collective_compute(
                "AllGather", mybir.AluOpType.bypass,
                replica_groups=g_batch,
                ins=[y_in_y.opt()],
                outs=[y_ag.opt()],
            )
            # own-token slice [768, 256] via rank-dependent column offset
            r4 = nc.gpsimd.partition_id() % 4
            yall = [yallp.tile([128, TS], F16, tag=f"ya{k}", name=f"ya{k}") for k in range(6)]
            for k in range(6):
                in_ap = bass.AP(
                    tensor=y_ag.tensor,
                    offset=r4 * TS + y_ag[k * 128, 0].offset,
                    ap=[[T, 128], [1, TS]],
                    dep_tracking_offset=y_ag[k * 128, 0].offset,
                )
                nc.gpsimd.dma_start(out=yall[k][:], in_=in_ap)

            if DEBUG_DUMP and layer == 0:
                nc.sync.dma_start(out=dbg[6][:, 0:TS], in_=yall[0][:])
                nc.sync.dma_start(out=dbg[7][:, 0:TS], in_=yall[5][:])
                yag_chk = hpool.tile([128, T], F16, tag="yagchk", name="yagchk")
                nc.sync.dma_start(out=yag_chk[:], in_=bass.AP(
                    tensor=y_ag.tensor, offset=y_ag[0, 0].offset,
                    ap=[[T, 128], [1, T]]))
                nc.sync.dma_start(out=dbg[3][:, :], in_=yag_chk[:])

            # ---- out-proj (own 256 tokens, full 768 contraction) ---------
            es_c = ExitStack()
            psO = es_c.enter_context(tc.tile_pool(name="psO", bufs=2, space="PSUM"))
            atpb_bc = bcast_row(lnrow, atpb[layer], 128, F16, E, tag="atpb")
            for t in range(2):
                po = psO.tile([128, E], F32, tag="o", name="o")
                for n0, n1 in ((0, 512), (512, 768)):
                    for k in range(6):
                        nc.tensor.matmul(po[:, n0:n1],
                                         yall[k][:, t * 128:(t + 1) * 128],
                                         watp_sb[k][:, n0:n1],
                                         start=(k == 0), stop=(k == 5))
                nc.vector.tensor_add(out=x_sb[t][:], in0=x_sb[t][:], in1=po[:])
                nc.vector.tensor_add(out=x_sb[t][:], in0=x_sb[t][:], in1=atpb_bc[:])
            if DEBUG_DUMP and layer == 0:
                for t in range(2):
                    xc = hpool.tile([128, E], F16, tag="ao", name="xc")
                    nc.vector.tensor_copy(xc[:], x_sb[t][:])
                    nc.sync.dma_start(out=dbg[8 + t][:, 0:E], in_=xc[:])
            es_c.close()

            # ---- LN2 + transpose -> h2T ---------------------------------
            ln2g_bc = bcast_row(lnrow, ln2g[layer], 128, F16, E, tag="ln2g")
            ln2b_bc = bcast_row(lnrow, ln2b[layer], 128, F16, E, tag="ln2b")
            es_t2 = ExitStack()
            psT2 = es_t2.enter_context(tc.tile_pool(name="psT2", bufs=4, space="PSUM"))
            h2T = [hTp.tile([128, TS], F16, tag=f"h2T{k}", name=f"h2T{k}") for k in range(6)]
            for t in range(2):
                h_t = hpool.tile([128, E], F16, tag="h", name="h")
                layernorm_t(x_sb[t][:], ln2g_bc, ln2b_bc, h_t)
                for k in range(6):
                    pt = psT2.tile([128, 128], F16, tag="tr2", name="tr2")
                    nc.tensor.transpose(pt[:], h_t[:, k * 128:(k + 1) * 128], ident[:])
                    dst = h2T[k][:, t * 128:(t + 1) * 128]
                    if k % 2 == 0:
                        nc.vector.tensor_copy(dst, pt[:])
                    else:
                        nc.scalar.activation(out=dst, in_=pt[:],
                                             func=mybir.ActivationFunctionType.Copy)
            es_t2.close()

            # ---- MLP fc: mT[m] = gelu(fcw[:,m].T @ h2T + fcb[m]) ---------
            es_d = ExitStack()
            psM = es_d.enter_context(tc.tile_pool(name="psM", bufs=4, space="PSUM"))
            psP = es_d.enter_context(tc.tile_pool(name="psP", bufs=1, space="PSUM"))
            mT = [mTp.tile([128, TS], F16, tag=f"mT{m}", name=f"mT{m}") for m in range(24)]
            if SIM_GELU:
                fcb17 = bias_p.tile([128, 24], F32, tag="fcb17", name="fcb17")
                nc.vector.tensor_scalar(out=fcb17[:], in0=fcb_sb[:], scalar1=1.702,
                                        scalar2=None,
                                        op0=mybir.AluOpType.mult)
            for m in range(24):
                ps = psM.tile([128, TS], F32, tag="m", name="m")
                for k in range(6):
                    nc.tensor.matmul(ps[:], fcw_sb[k][:, m * 128:(m + 1) * 128],
                                     h2T[k][:], start=(k == 0), stop=(k == 5))
                if SIM_GELU:
                    sgm = mTp.tile([128, TS], F32, tag="sgm", name="sgm")
                    nc.scalar.activation(out=sgm[:], in_=ps[:],
                                         func=mybir.ActivationFunctionType.Sigmoid,
                                         scale=1.702, bias=fcb17[:, m:m + 1])
                    nc.vector.tensor_scalar_add(out=mT[m][:], in0=ps[:],
                                                scalar1=fcb_sb[:, m:m + 1])
                    nc.vector.tensor_mul(out=mT[m][:], in0=mT[m][:], in1=sgm[:])
                else:
                    nc.scalar.activation(out=mT[m][:], in_=ps[:],
                                         func=mybir.ActivationFunctionType.Gelu_apprx_tanh,
                                         bias=fcb_sb[:, m:m + 1])

            # ---- MLP pr + residual --------------------------------------
            prb_bc = bcast_row(lnrow, prb[layer], 128, F16, E, tag="prb")
            ps2 = [psP.tile([128, E], F32, tag=f"p{t}", name=f"p{t}") for t in range(2)]
            for m in range(24):
                for t in range(2):
                    for n0, n1 in ((0, 512), (512, 768)):
                        nc.tensor.matmul(ps2[t][:, n0:n1],
                                         mT[m][:, t * 128:(t + 1) * 128],
                                         prw_sb[m][:, n0:n1],
                                         start=(m == 0), stop=(m == 23))
            for t in range(2):
                nc.vector.tensor_add(out=x_sb[t][:], in0=x_sb[t][:], in1=ps2[t][:])
                nc.vector.tensor_add(out=x_sb[t][:], in0=x_sb[t][:], in1=prb_bc[:])
            es_d.close()

        # ---- final LN + AllGather(all 8) + lm_head -----------------------
        lnfg_bc = bcast_row(lnrow, lnfg[0], 128, F16, E, tag="lnfg")
        lnfb_bc = bcast_row(lnrow, lnfb[0], 128, F16, E, tag="lnfb")
        es_tf = ExitStack()
        psTf = es_tf.enter_context(tc.tile_pool(name="psTf", bufs=4, space="PSUM"))
        xfT = [hTp.tile([128, TS], F16, tag=f"hT{k}", name=f"xfT{k}") for k in range(6)]
        for t in range(2):
            h_t = hpool.tile([128, E], F16, tag="h", name="h")
            layernorm_t(x_sb[t][:], lnfg_bc, lnfb_bc, h_t)
            for k in range(6):
                pt = psTf.tile([128, 128], F16, tag="trf", name="trf")
                nc.tensor.transpose(pt[:], h_t[:, k * 128:(k + 1) * 128], ident[:])
                dst = xfT[k][:, t * 128:(t + 1) * 128]
                if k % 2 == 0:
                    nc.vector.tensor_copy(dst, pt[:])
                else:
                    nc.scalar.activation(out=dst, in_=pt[:],
                                         func=mybir.ActivationFunctionType.Copy)
        es_tf.close()
        for half in range(2):
            for k in range(3):
                nc.sync.dma_start(out=xf_in[half][k * 128:(k + 1) * 128, :],
                                  in_=xfT[half * 3 + k][:])
            nc.gpsimd.collective_compute(
                "AllGather", mybir.AluOpType.bypass,
                replica_groups=g_all,
                ins=[xf_in[half].opt()],
                outs=[xf_ag[half].opt()],
            )
        es_l.close()
        es_h = es.enter_context(ExitStack())
        xfp = es_h.enter_context(tc.tile_pool(name="xfp", bufs=1))
        wtep = es_h.enter_context(tc.tile_pool(name="wtep", bufs=2))
        lop = es_h.enter_context(tc.tile_pool(name="lop", bufs=4))
        psL = es_h.enter_context(tc.tile_pool(name="psL", bufs=1, space="PSUM"))

        # xfT_full 6 x [128, 2048] fp16 (3D-AP load across 8 ranks)
        xf_full = [xfp.tile([128, B * T], F16, tag=f"xf{k}", name=f"xf{k}") for k in range(6)]
        for k in range(6):
            half, kk = divmod(k, 3)
            src_t = xf_ag[half]
            in_ap = bass.AP(
                tensor=src_t.tensor,
                offset=src_t[kk * 128, 0].offset,
                ap=[[TS, 128], [3 * 128 * TS, NC], [1, TS]],
            )
            eng = (nc.sync, nc.scalar, nc.gpsimd)[k % 3]
            eng.dma_start(out=xf_full[k][:].rearrange("p (r t) -> p r t", r=NC),
                          in_=in_ap)

        nch = (VS + 511) // 512
        for n in range(nch):
            n0 = n * 512
            nw = min(512, VS - n0)
            wte_sb = [wtep.tile([128, 512], F16, tag=f"wte{k}", name=f"wte{k}") for k in range(6)]
            for k in range(6):
                nc.gpsimd.dma_start(out=wte_sb[k][:, 0:nw],
                                    in_=wteT[n, k, :, 0:nw])
            for th in range(2):
                pss = [psL.tile([128, 512], F32, tag=f"l{t}", name=f"l{t}") for t in range(8)]
                for k in range(6):
                    for t in range(8):
                        nc.tensor.matmul(pss[t][:, 0:nw],
                                         xf_full[k][:, (th * 8 + t) * 128:(th * 8 + t + 1) * 128],
                                         wte_sb[k][:, 0:nw],
                                         start=(k == 0), stop=(k == 5))
                for t in range(8):
                    lo = lop.tile([128, 512], F16, tag="lo", name="lo")
                    if t % 2 == 0:
                        nc.vector.tensor_copy(lo[:, 0:nw], pss[t][:, 0:nw])
                    else:
                        nc.scalar.activation(out=lo[:, 0:nw], in_=pss[t][:, 0:nw],
                                             func=mybir.ActivationFunctionType.Copy)
                    eng = nc.sync if t % 2 == 0 else nc.scalar
                    row0 = (n * 16 + th * 8 + t) * 128
                    eng.dma_start(out=logits[row0:row0 + 128, 0:nw],
                                  in_=lo[:, 0:nw])

    nc.compile()
    return nc


def _block_wte(wt, nch, vs_pad):
    # [768, VS] -> [nch, 6, 128, 512] fp16 blocked
    pad = np.zeros((E, vs_pad - wt.shape[1]), np.float32)
    wtp = np.concatenate([wt, pad], axis=1)
    return np.ascontiguousarray(
        wtp.reshape(6, 128, nch, 512).transpose(2, 0, 1, 3).astype(np.float16))


def _prep_inputs(idx, wte, wpe, ln1_w, ln1_b, attn_w, attn_b, atp_w, atp_b,
                 ln2_w, ln2_b, fc_w, fc_b, pr_w, pr_b, lnf_w, lnf_b):
    idx = np.asarray(idx)
    f = lambda a: np.ascontiguousarray(np.asarray(a), dtype=np.float32)
    h = lambda a: np.ascontiguousarray(np.asarray(a), dtype=np.float16)
    wte, wpe = f(wte), f(wpe)
    x0 = wte[idx.reshape(-1)] + np.tile(wpe[:T], (B, 1))  # [2048, 768]
    wte_pad = np.zeros((VPAD, E), np.float32)
    wte_pad[:V] = wte
    wteT_full = np.ascontiguousarray(wte_pad.T)  # [768, VPAD]
    nch = (VS + 511) // 512
    vs_pad = nch * 512

    attn_w, attn_b = f(attn_w), f(attn_b)
    atp_w, atp_b = f(atp_w), f(atp_b)
    fc_w, fc_b, pr_w, pr_b = f(fc_w), f(fc_b), f(pr_w), f(pr_b)

    # fold v-bias through atp: y_true = y/den + bv  ->  + bv @ atp_w
    bv_full = attn_b[:, 2 * E:]                       # [L, 768]
    atpb_eff = atp_b + np.einsum('le,leo->lo', bv_full, atp_w)

    in_maps = []
    for c in range(NC):
        hs = 3 * (c % 4)
        q = [attn_w[:, :, (hs + hh) * HD:(hs + hh + 1) * HD] for hh in range(3)]
        k = [attn_w[:, :, E + (hs + hh) * HD:E + (hs + hh + 1) * HD] for hh in range(3)]
        v = [attn_w[:, :, 2 * E + (hs + hh) * HD:2 * E + (hs + hh + 1) * HD] for hh in range(3)]
        pad = np.zeros((L, E, HD), np.float32)
        # cols: [q0|q1, k0|k1, pad|q2, pad|k2]
        wqk_c = np.concatenate([q[0], q[1], k[0], k[1], pad, q[2], pad, k[2]], axis=2)
        qb = [attn_b[:, (hs + hh) * HD:(hs + hh + 1) * HD] for hh in range(3)]
        kb = [attn_b[:, E + (hs + hh) * HD:E + (hs + hh + 1) * HD] for hh in range(3)]
        zb = np.zeros((L, HD), np.float32)
        bqk_c = np.stack([
            np.concatenate([qb[0], qb[1]], axis=1),
            np.concatenate([kb[0], kb[1]], axis=1),
            np.concatenate([zb, qb[2]], axis=1),
            np.concatenate([zb, kb[2]], axis=1),
        ], axis=2)  # [L, 128, 4]
        wv_c = np.concatenate(v, axis=2)
        in_maps.append({
            "x0s": np.ascontiguousarray(x0[c * TS:(c + 1) * TS]),
            "wqk": h(wqk_c), "bqk": np.ascontiguousarray(bqk_c),
            "wv": h(wv_c),
            "watp": h(atp_w),
            "atpb": np.ascontiguousarray(atpb_eff),
            "fcw": h(fc_w), "fcb": np.ascontiguousarray(
                fc_b.reshape(L, 24, 128).transpose(0, 2, 1)),
            "prw": h(pr_w), "prb": pr_b,
            "ln1g": f(ln1_w), "ln1b": f(ln1_b),
            "ln2g": f(ln2_w), "ln2b": f(ln2_b),
            "lnfg": f(lnf_w).reshape(1, E), "lnfb": f(lnf_b).reshape(1, E),
            "wteT": _block_wte(wteT_full[:, c * VS:(c + 1) * VS], nch, vs_pad),
        })
    return in_maps


def kernel(trace=False, **inputs):
    if "nc" not in _CACHE:
        _CACHE["nc"] = _build_program()
    nc = _CACHE["nc"]
    in_maps = _prep_inputs(**inputs)
    res = run_bass_kernel_spmd(nc, in_maps, core_ids=list(range(NC)), trace=trace)
    _CACHE["last_result"] = res
    nch = (VS + 511) // 512
    full = np.empty((B * T, V), np.float32)
    for c in range(NC):
        blk = res.results[c]["logits"].reshape(nch, 16 * 128, 512)
        for n in range(nch):
            n0 = c * VS + n * 512
            nw = min(512, VS - n * 512)
            lo = blk[n][:, :nw]
            v0 = min(n0, V)
            v1 = min(n0 + nw, V)
            if v1 > v0:
                full[:, v0:v1] = lo[:, :v1 - v0]
    return full.reshape(B, T, V)


# revision 37
# speedup vs baseline: 1.0474x; 1.0474x over previous
"""GPT (4-layer, E=768, H=12, T=1024, B=2, V=50257) forward on 8 trn2 cores.

Sharding:
  - Residual stream x token-sharded fp32: core c owns tokens [c*256,(c+1)*256)
    of the flattened [2048] (batch-major): cores 0-3 = batch 0, 4-7 = batch 1.
  - Attention head-sharded within each batch group of 4 cores (3 heads each):
    AllGather hidden (fp16, split in 2 halves for overlap), compute q/k/v +
    scores + y for my heads over all 1024 tokens, then AllToAll the normalized
    per-head y back to token owners (uniform SPMD), out-proj token-local with
    full 768 contraction (fp32 psum, no low-precision reduction anywhere).
  - MLP fully token-local: fc weight-stationary (out [hid,tok]),
    pr activation-stationary (out [tok,E]) - no transposes inside MLP.
  - lm_head vocab-sharded fp16: AllGather lnf(x) (all 8), each core computes
    [2048, 6284] logit slice, vocab-group-outer loop with resident xfT.
  - All matmul inputs fp16 (fp32 psum accumulate); scores 2-head row-packed
    (tile_position concurrency); h2 on row-group 64-127.
"""

import sys
from contextlib import ExitStack
import numpy as np

sys.path.insert(0, "/opt/trn_rl_repo")

import concourse.bass as bass
import concourse.mybir as mybir
import concourse.tile as tile
from concourse import bacc
from concourse.bass_utils import run_bass_kernel_spmd
from concourse.masks import make_identity

L, H, E, T, V = 4, 12, 768, 1024, 50257
B = 2
NC = 8
TS = (B * T) // NC          # 256 tokens per core
VS = 6284                   # vocab slice per core (padded V = 50272)
VPAD = VS * NC
HD = 64
EPS = 1e-5
SCALE = float(1.0 / np.sqrt(np.float32(E)))
F32 = mybir.dt.float32
F16 = mybir.dt.float16

L_RUN = L  # layers actually executed (tests may truncate)
SIM_GELU = False  # sim lacks Gelu_apprx_tanh; use x*sigmoid(1.702x) for debug
DEBUG_DUMP = False  # dump layer-0 intermediates to a "dbg" output
_CACHE = {}


def _build_program():
    nc = bacc.Bacc("TRN2", target_bir_lowering=False, debug=False, num_devices=NC)

    # ---- I/O -------------------------------------------------------------
    x0s = nc.dram_tensor("x0s", [TS, E], F32, kind="ExternalInput")
    # wqk cols: [q0|q1 (128), k0|k1 (128), pad|q2 (128), pad|k2 (128)]
    wqk = nc.dram_tensor("wqk", [L, E, 512], F16, kind="ExternalInput")
    bqk = nc.dram_tensor("bqk", [L, 128, 4], F32, kind="ExternalInput")
    wv = nc.dram_tensor("wv", [L, E, 3 * HD], F16, kind="ExternalInput")
    watp = nc.dram_tensor("watp", [L, E, E], F16, kind="ExternalInput")  # full (head-major rows)
    atpb = nc.dram_tensor("atpb", [L, E], F32, kind="ExternalInput")     # includes bv@watp fold
    fcw = nc.dram_tensor("fcw", [L, E, 4 * E], F16, kind="ExternalInput")
    fcb = nc.dram_tensor("fcb", [L, 128, 24], F32, kind="ExternalInput")
    prw = nc.dram_tensor("prw", [L, 4 * E, E], F16, kind="ExternalInput")
    prb = nc.dram_tensor("prb", [L, E], F32, kind="ExternalInput")
    ln1g = nc.dram_tensor("ln1g", [L, E], F32, kind="ExternalInput")
    ln1b = nc.dram_tensor("ln1b", [L, E], F32, kind="ExternalInput")
    ln2g = nc.dram_tensor("ln2g", [L, E], F32, kind="ExternalInput")
    ln2b = nc.dram_tensor("ln2b", [L, E], F32, kind="ExternalInput")
    lnfg = nc.dram_tensor("lnfg", [1, E], F32, kind="ExternalInput")
    lnfb = nc.dram_tensor("lnfb", [1, E], F32, kind="ExternalInput")
    wteT = nc.dram_tensor("wteT", [(VS + 511) // 512, 6, 128, 512], F16, kind="ExternalInput")
    NCH = (VS + 511) // 512
    logits = nc.dram_tensor("logits", [NCH * 16 * 128, 512], F16, kind="ExternalOutput")
    dbg = (nc.dram_tensor("dbg", [12, 128, T], F16, kind="ExternalOutput")
           if DEBUG_DUMP else None)

    tri_np = (np.arange(128)[None, :] >= np.arange(128)[:, None]).astype(np.float16)
    tri = nc.inline_tensor(tri_np, name="tri_const")

    g_all = [list(range(NC))]
    g_batch = [[0, 1, 2, 3], [4, 5, 6, 7]]

    def bcast_row(pool, src_ap, n, dtype, w, tag=None):
        """Replicate a [w] DRAM row across n partitions via broadcast DMA."""
        t = pool.tile([n, w], dtype, tag=tag)
        in_ap = bass.AP(
            tensor=src_ap.tensor,
            offset=src_ap.offset,
            ap=[[0, n]] + [list(p) for p in src_ap.ap],
        )
        eng = nc.gpsimd if dtype != src_ap.dtype else nc.sync
        eng.dma_start(out=t[:], in_=in_ap)
        return t

    with tile.TileContext(nc) as tc, ExitStack() as es:
        const = es.enter_context(tc.tile_pool(name="const", bufs=1))
        xp = es.enter_context(tc.tile_pool(name="xp", bufs=1))
        lnrow = es.enter_context(tc.tile_pool(name="lnrow", bufs=1))
        stat = es.enter_context(tc.tile_pool(name="stat", bufs=2))
        hpool = es.enter_context(tc.tile_pool(name="hpool", bufs=2))
        dram = es.enter_context(tc.tile_pool(name="dram", bufs=1, space="DRAM"))

        ident_f = const.tile([128, 128], F32, name="ident_f")
        make_identity(nc, ident_f)
        ident = const.tile([128, 128], F16, name="ident")
        nc.vector.tensor_copy(ident[:], ident_f[:])
        tri_sb = const.tile([128, 128], F16, name="tri_sb")
        nc.sync.dma_start(out=tri_sb[:], in_=tri[:, :])
        eps_sb = const.tile([128, 1], F32, name="eps_sb")
        nc.vector.memset(eps_sb, EPS)
        ones3 = const.tile([128, 3], F16, name="ones3")
        nc.vector.memset(ones3, 1.0)

        # persistent residual stream [256, 768] fp32 as two [128, 768] tiles
        x_sb = [xp.tile([128, E], F32, tag=f"x{t}", name=f"x{t}") for t in range(2)]
        for t in range(2):
            nc.sync.dma_start(out=x_sb[t][:], in_=x0s[t * 128:(t + 1) * 128, :])

        # DRAM bounce buffers for collectives (fp16)
        hT_in = [dram.tile([3 * 128, TS], F16, name=f"hT_in{i}") for i in range(2)]
        hT_ag = [dram.tile([4 * 3 * 128, TS], F16, name=f"hT_ag{i}") for i in range(2)]
        y_in_y = dram.tile([3 * HD, T], F16, name="y_in_y")
        y_ag = dram.tile([4 * 3 * HD, T], F16, name="y_ag")
        xf_in = [dram.tile([3 * 128, TS], F16, name=f"xf_in{i}") for i in range(2)]
        xf_ag = [dram.tile([NC * 3 * 128, TS], F16, name=f"xf_ag{i}") for i in range(2)]

        def layernorm_t(x_ap, g_bc, b_bc, out_tile):
            """LN over free dim (768) of [128, 768] fp32 tile -> out fp16."""
            stats = stat.tile([128, 3, 6], F32, tag="bn_stats", name="bn_stats_t")
            xr = x_ap.rearrange("p (s d) -> p s d", s=3)
            for s in range(3):
                nc.vector.bn_stats(out=stats[:, s, :], in_=xr[:, s, :])
            mv = stat.tile([128, 2], F32, tag="bn_aggr", name="bn_aggr_t")
            nc.vector.bn_aggr(out=mv[:], in_=stats[:])
            rstd = stat.tile([128, 1], F32, tag="rstd", name="rstd_t")
            nc.scalar.activation(out=rstd[:], in_=mv[:, 1:2],
                                 func=mybir.ActivationFunctionType.Sqrt,
                                 bias=eps_sb[:], scale=1.0)
            nc.vector.reciprocal(out=rstd[:], in_=rstd[:])
            tmp = stat.tile([128, E], F32, tag="ln_tmp", name="ln_tmp")
            nc.vector.tensor_scalar(out=tmp[:], in0=x_ap,
                                    scalar1=mv[:, 0:1], scalar2=rstd[:],
                                    op0=mybir.AluOpType.subtract,
                                    op1=mybir.AluOpType.mult)
            nc.vector.tensor_mul(out=tmp[:], in0=tmp[:], in1=g_bc[:])
            nc.vector.tensor_add(out=out_tile[:], in0=tmp[:], in1=b_bc[:])

        # ---- persistent layer pools (tags reused across layers) ----------
        es_l = es.enter_context(ExitStack())
        wqkp = es_l.enter_context(tc.tile_pool(name="wqkp", bufs=1))
        wvp = es_l.enter_context(tc.tile_pool(name="wvp", bufs=1))
        watpp = es_l.enter_context(tc.tile_pool(name="watpp", bufs=1))
        fcwp = es_l.enter_context(tc.tile_pool(name="fcwp", bufs=1))
        prwp = es_l.enter_context(tc.tile_pool(name="prwp", bufs=1))
        bias_p = es_l.enter_context(tc.tile_pool(name="bias_p", bufs=2))
        hTp = es_l.enter_context(tc.tile_pool(name="hTp", bufs=1))
        hTbp = es_l.enter_context(tc.tile_pool(name="hTbp", bufs=1))
        qkp = es_l.enter_context(tc.tile_pool(name="qkp", bufs=1))
        vp = es_l.enter_context(tc.tile_pool(name="vp", bufs=1))
        ep = es_l.enter_context(tc.tile_pool(name="ep", bufs=2))
        yp = es_l.enter_context(tc.tile_pool(name="yp", bufs=1))
        sm = es_l.enter_context(tc.tile_pool(name="sm", bufs=2))
        mTp = es_l.enter_context(tc.tile_pool(name="mTp", bufs=1))
        yallp = es_l.enter_context(tc.tile_pool(name="yallp", bufs=1))

        # v_sb layout [128, 258]: h0 [v0|1] at 0:65, h1 [1|pad63|v1] at
        # 65:193 (den at psum row 0, y1 at rows 64-127 for packed
        # out-proj), h2 [v2|1] at 193:258. Constant cols written once.
        v_sb = [vp.tile([128, 258], F16, tag=f"v{t}", name=f"v{t}") for t in range(8)]
        for t in range(8):
            nc.vector.tensor_copy(v_sb[t][:, 64:65], ones3[:, 0:1])
            nc.vector.tensor_copy(v_sb[t][:, 65:66], ones3[:, 1:2])
            nc.vector.tensor_copy(v_sb[t][:, 257:258], ones3[:, 2:3])
            nc.vector.memset(v_sb[t][:, 66:129], 0.0)

        for layer in range(L_RUN):
            # ---- weight loads (gpsimd queue; Tile schedules early) ------
            wqk_sb = [wqkp.tile([128, 512], F16, tag=f"wqk{k}", name=f"wqk{k}") for k in range(6)]
            wv_sb = [wvp.tile([128, 192], F16, tag=f"wv{k}", name=f"wv{k}") for k in range(6)]
            bqk_sb = bias_p.tile([128, 4], F32, tag="bqk", name="bqk")
            nc.sync.dma_start(out=bqk_sb[:], in_=bqk[layer])
            fcb_sb = bias_p.tile([128, 24], F32, tag="fcb", name="fcb")
            nc.sync.dma_start(out=fcb_sb[:], in_=fcb[layer])
            ln1g_bc = bcast_row(lnrow, ln1g[layer], 128, F16, E, tag="ln1g")
            ln1b_bc = bcast_row(lnrow, ln1b[layer], 128, F16, E, tag="ln1b")

            # ---- LN1 + transpose -> hT [768, 256] fp16 -------------------
            es_t1 = ExitStack()
            psT = es_t1.enter_context(tc.tile_pool(name="psT", bufs=4, space="PSUM"))
            hT = [hTp.tile([128, TS], F16, tag=f"hT{k}", name=f"hT{k}") for k in range(6)]
            for t in range(2):
                h_t = hpool.tile([128, E], F16, tag="h", name="h")
                layernorm_t(x_sb[t][:], ln1g_bc, ln1b_bc, h_t)
                for k in range(6):
                    pt = psT.tile([128, 128], F16, tag="tr", name="tr")
                    nc.tensor.transpose(pt[:], h_t[:, k * 128:(k + 1) * 128], ident[:])
                    dst = hT[k][:, t * 128:(t + 1) * 128]
                    if k % 2 == 0:
                        nc.vector.tensor_copy(dst, pt[:])
                    else:
                        nc.scalar.activation(out=dst, in_=pt[:],
                                             func=mybir.ActivationFunctionType.Copy)
            es_t1.close()

            # ---- AllGather hidden in 2 halves (within batch group of 4) --
            for half in range(2):
                for k in range(3):
                    nc.sync.dma_start(out=hT_in[half][k * 128:(k + 1) * 128, :],
                                      in_=hT[half * 3 + k][:])
                nc.gpsimd.collective_compute(
                    "AllGather", mybir.AluOpType.bypass,
                    replica_groups=g_batch,
                    ins=[hT_in[half].opt()],
                    outs=[hT_ag[half].opt()],
                )
            # load hTb 6 x [128, 1024] fp16 (one 3D-AP DMA per k-chunk)
            hTb = [hTbp.tile([128, T], F16, tag=f"hTb{k}", name=f"hTb{k}") for k in range(6)]
            for k in range(6):
                half, kk = divmod(k, 3)
                src = hT_ag[half]
                in_ap = bass.AP(
                    tensor=src.tensor,
                    offset=src[kk * 128, 0].offset,
                    ap=[[TS, 128], [3 * 128 * TS, 4], [1, TS]],
                )
                eng = (nc.sync, nc.scalar)[k % 2]
                eng.dma_start(
                    out=hTb[k][:].rearrange("p (r t) -> p r t", r=4),
                    in_=in_ap)

            if DEBUG_DUMP and layer == 0:
                nc.sync.dma_start(out=dbg[0], in_=hTb[0][:])
                nc.sync.dma_start(out=dbg[11][:, 0:128], in_=tri_sb[:])

            # weight loads on gpsimd AFTER collective triggers (in-order queue)
            watp_sb = [watpp.tile([128, E], F16, tag=f"wa{k}", name=f"wa{k}") for k in range(6)]
            for k in range(6):
                nc.gpsimd.dma_start(out=wqk_sb[k][:], in_=wqk[layer, k * 128:(k + 1) * 128, :])
                nc.gpsimd.dma_start(out=wv_sb[k][:], in_=wv[layer, k * 128:(k + 1) * 128, :])
            for k in range(6):
                nc.gpsimd.dma_start(out=watp_sb[k][:], in_=watp[layer, k * 128:(k + 1) * 128, :])
            fcw_sb = [fcwp.tile([128, 4 * E], F16, tag=f"fcw{k}", name=f"fcw{k}") for k in range(6)]
            for k in range(6):
                nc.gpsimd.dma_start(out=fcw_sb[k][:], in_=fcw[layer, k * 128:(k + 1) * 128, :])
            prw_sb = [prwp.tile([128, E], F16, tag=f"prw{m}", name=f"prw{m}") for m in range(24)]
            for m in range(24):
                nc.gpsimd.dma_start(out=prw_sb[m][:], in_=prw[layer, m * 128:(m + 1) * 128, :])

            # ---- QKV ----------------------------------------------------
            es_a = ExitStack()
            psQK = es_a.enter_context(tc.tile_pool(name="psQK", bufs=3, space="PSUM"))
            psV = es_a.enter_context(tc.tile_pool(name="psV", bufs=2, space="PSUM"))
            # qT2/kT2: rows 0-63 head0, 64-127 head1; q1T/k1T rows 64-127 head2
            qT2 = qkp.tile([128, T], F16, tag="qT2", name="qT2")
            kT2 = qkp.tile([128, T], F16, tag="kT2", name="kT2")
            q1T = qkp.tile([128, T], F16, tag="q1T", name="q1T")
            k1T = qkp.tile([128, T], F16, tag="k1T", name="k1T")
            qk_dsts = [qT2, kT2, q1T, k1T]
            for s in range(4):
                ps = psQK.tile([128, T], F32, tag="qk", name="qk")
                for n in range(2):
                    for k in range(6):
                        nc.tensor.matmul(ps[:, n * 512:(n + 1) * 512],
                                         wqk_sb[k][:, s * 128:(s + 1) * 128],
                                         hTb[k][:, n * 512:(n + 1) * 512],
                                         start=(k == 0), stop=(k == 5))
                if s < 2:
                    nc.vector.tensor_scalar_add(out=qk_dsts[s][:], in0=ps[:],
                                                scalar1=bqk_sb[:, s:s + 1])
                else:
                    nc.vector.tensor_scalar_add(out=qk_dsts[s][64:128, :],
                                                in0=ps[64:128, :],
                                                scalar1=bqk_sb[64:128, s:s + 1])
            for t in range(8):
                ps = psV.tile([128, 192], F32, tag="v", name="v")
                for k in range(6):
                    nc.tensor.matmul(ps[:], hTb[k][:, t * 128:(t + 1) * 128],
                                     wv_sb[k][:], start=(k == 0), stop=(k == 5))
                if t % 2 == 0:
                    nc.vector.tensor_copy(v_sb[t][:, 0:64], ps[:, 0:64])
                    nc.vector.tensor_copy(v_sb[t][:, 129:193], ps[:, 64:128])
                    nc.vector.tensor_copy(v_sb[t][:, 193:257], ps[:, 128:192])
                else:
                    nc.scalar.activation(out=v_sb[t][:, 0:64], in_=ps[:, 0:64],
                                         func=mybir.ActivationFunctionType.Copy)
                    nc.scalar.activation(out=v_sb[t][:, 129:193], in_=ps[:, 64:128],
                                         func=mybir.ActivationFunctionType.Copy)
                    nc.scalar.activation(out=v_sb[t][:, 193:257], in_=ps[:, 128:192],
                                         func=mybir.ActivationFunctionType.Copy)

            if DEBUG_DUMP and layer == 0:
                nc.sync.dma_start(out=dbg[1], in_=qT2[:])
                nc.sync.dma_start(out=dbg[2], in_=kT2[:])
                nc.sync.dma_start(out=dbg[3], in_=k1T[:])
                nc.sync.dma_start(out=dbg[10][:, 0:258], in_=v_sb[0][:])

            # ---- attention: scores + y, head 0/1 row-packed --------------
            es_a.close()
            es_b = ExitStack()
            psS = es_b.enter_context(tc.tile_pool(name="psS", bufs=2, space="PSUM"))
            psY = es_b.enter_context(tc.tile_pool(name="psY", bufs=1, space="PSUM"))
            # h0: y rows 0:64, den 64; h1: den 0, y 64:128; h2: y 0:64, den 64
            yps0 = psY.tile([65, T], F32, tag="y0", name="y0")
            yps1 = psY.tile([128, T], F32, tag="y1", name="y1")
            yps2 = psY.tile([65, T], F32, tag="y2", name="y2")
            heads = [(qT2[0:64, :], kT2[0:64, :], yps0, 0, 65),
                     (qT2[64:128, :], kT2[64:128, :], yps1, 65, 193),
                     (q1T[64:128, :], k1T[64:128, :], yps2, 193, 258)]
            for j in range(8):
                qs = j * 128
                qlen = T - qs
                for h in range(3):
                    qT_h, kT_h, yout, v0, v1 = heads[h]
                    e_sb = ep.tile([128, T], F16, tag=f"e{h}", name=f"e{h}")
                    off = 0
                    while off < qlen:
                        cl = min(512, qlen - off)
                        pss = psS.tile([128, 512], F32, tag="s", name="s")
                        nc.tensor.matmul(pss[:, 0:cl], kT_h[:, qs:qs + 128],
                                         qT_h[:, qs + off: qs + off + cl],
                                         start=True, stop=True)
                        nc.scalar.activation(out=e_sb[:, off:off + cl],
                                             in_=pss[:, 0:cl],
                                             func=mybir.ActivationFunctionType.Exp,
                                             scale=SCALE)
                        off += cl
                    nc.vector.tensor_mul(out=e_sb[:, 0:128], in0=e_sb[:, 0:128],
                                         in1=tri_sb[:])
                    if qs < 512:
                        nc.tensor.matmul(yout[:, qs:512],
                                         v_sb[j][:, v0:v1],
                                         e_sb[:, 0:512 - qs],
                                         start=(j == 0), stop=(j == 3))
                    nc.tensor.matmul(yout[:, max(qs, 512):T],
                                     v_sb[j][:, v0:v1],
                                     e_sb[:, max(qs, 512) - qs:qlen],
                                     start=(j == 0), stop=(j == 7))

            # ---- normalize into packed y tiles ---------------------------
            # y01T rows 0:64 = head0, 64:128 = head1; y2T rows 0:64 = head2
            y01T = yp.tile([128, T], F16, tag="y01T", name="y01T")
            y2T = yp.tile([64, T], F16, tag="y2T", name="y2T")
            for h, (yout, den_row, dst) in enumerate(
                    [(yps0, 64, y01T[0:64, :]), (yps1, 0, y01T[64:128, :]),
                     (yps2, 64, y2T[:])]):
                # den psum->sbuf (gpsimd cannot read PSUM), broadcast to all
                # 128 partitions, then one DVE divide
                den_sb = sm.tile([1, T], F32, tag="den", name="den")
                nc.vector.tensor_copy(den_sb[:], yout[den_row:den_row + 1, :])
                bcf = sm.tile([128, T], F32, tag="bcf", name="bcf")
                nc.gpsimd.partition_broadcast(bcf[:], den_sb[:])
                lo = 64 if h == 1 else 0
                ysrc = yout[64:128, :] if h == 1 else yout[0:64, :]
                nc.vector.reciprocal_approx_fast(out=bcf[:], in_=bcf[:])
                nc.vector.tensor_mul(out=dst, in0=ysrc, in1=bcf[lo:lo + 64, :])
            if DEBUG_DUMP and layer == 0:
                nc.sync.dma_start(out=dbg[4], in_=y01T[:])
                nc.sync.dma_start(out=dbg[5][0:64, :], in_=y2T[:])
            es_b.close()

            # ---- AllGather y (all heads, all tokens) ---------------------
            nc.sync.dma_start(out=y_in_y[0:128, :], in_=y01T[:])
            nc.sync.dma_start(out=y_in_y[128:192, :], in_=y2T[:])
            nc.gpsimd.collective_compute(
                "AllGather", mybir.AluOpType.bypass,
                replica_groups=g_batch,
                ins=[y_in_y.opt()],
                outs=[y_ag.opt()],
            )
            # own-token slice [768, 256] via rank-dependent column offset
            r4 = nc.gpsimd.partition_id() % 4
            yall = [yallp.tile([128, TS], F16, tag=f"ya{k}", name=f"ya{k}") for k in range(6)]
            for k in range(6):
                in_ap = bass.AP(
                    tensor=y_ag.tensor,
                    offset=r4 * TS + y_ag[k * 128, 0].offset,
                    ap=[[T, 128], [1, TS]],
                    dep_tracking_offset=y_ag[k * 128, 0].offset,
                )
                nc.gpsimd.dma_start(out=yall[k][:], in_=in_ap)

            if DEBUG_DUMP and layer == 0:
                nc.sync.dma_start(out=dbg[6][:, 0:TS], in_=yall[0][:])
                nc.sync.dma_start(out=dbg[7][:, 0:TS], in_=yall[5][:])
                yag_chk = hpool.tile([128, T], F16, tag="yagchk", name="yagchk")
                nc.sync.dma_start(out=yag_chk[:], in_=bass.AP(
                    tensor=y_ag.tensor, offset=y_ag[0, 0].offset,
                    ap=[[T, 128], [1, T]]))
                nc.sync.dma_start(out=dbg[3][:, :], in_=yag_chk[:])

            # ---- out-proj (own 256 tokens, full 768 contraction) ---------
            es_c = ExitStack()
            psO = es_c.enter_context(tc.tile_pool(name="psO", bufs=2, space="PSUM"))
            atpb_bc = bcast_row(lnrow, atpb[layer], 128, F16, E, tag="atpb")
            for t in range(2):
                po = psO.tile([128, E], F32, tag="o", name="o")
                for n0, n1 in ((0, 512), (512, 768)):
                    for k in range(6):
                        nc.tensor.matmul(po[:, n0:n1],
                                         yall[k][:, t * 128:(t + 1) * 128],
                                         watp_sb[k][:, n0:n1],
                                         start=(k == 0), stop=(k == 5))
                nc.vector.tensor_add(out=x_sb[t][:], in0=x_sb[t][:], in1=po[:])
                nc.vector.tensor_add(out=x_sb[t][:], in0=x_sb[t][:], in1=atpb_bc[:])
            if DEBUG_DUMP and layer == 0:
                for t in range(2):
                    xc = hpool.tile([128, E], F16, tag="ao", name="xc")
                    nc.vector.tensor_copy(xc[:], x_sb[t][:])
                    nc.sync.dma_start(out=dbg[8 + t][:, 0:E], in_=xc[:])
            es_c.close()

            # ---- LN2 + transpose -> h2T ---------------------------------
            ln2g_bc = bcast_row(lnrow, ln2g[layer], 128, F16, E, tag="ln2g")
            ln2b_bc = bcast_row(lnrow, ln2b[layer], 128, F16, E, tag="ln2b")
            es_t2 = ExitStack()
            psT2 = es_t2.enter_context(tc.tile_pool(name="psT2", bufs=4, space="PSUM"))
            h2T = [hTp.tile([128, TS], F16, tag=f"h2T{k}", name=f"h2T{k}") for k in range(6)]
            for t in range(2):
                h_t = hpool.tile([128, E], F16, tag="h", name="h")
                layernorm_t(x_sb[t][:], ln2g_bc, ln2b_bc, h_t)
                for k in range(6):
                    pt = psT2.tile([128, 128], F16, tag="tr2", name="tr2")
                    nc.tensor.transpose(pt[:], h_t[:, k * 128:(k + 1) * 128], ident[:])
                    dst = h2T[k][:, t * 128:(t + 1) * 128]
                    if k % 2 == 0:
                        nc.vector.tensor_copy(dst, pt[:])
                    else:
                        nc.scalar.activation(out=dst, in_=pt[:],
                                             func=mybir.ActivationFunctionType.Copy)
            es_t2.close()

            # ---- MLP fc: mT[m] = gelu(fcw[:,m].T @ h2T + fcb[m]) ---------
            es_d = ExitStack()
            psM = es_d.enter_context(tc.tile_pool(name="psM", bufs=4, space="PSUM"))
            psP = es_d.enter_context(tc.tile_pool(name="psP", bufs=1, space="PSUM"))
            mT = [mTp.tile([128, TS], F16, tag=f"mT{m}", name=f"mT{m}") for m in range(24)]
            if SIM_GELU:
                fcb17 = bias_p.tile([128, 24], F32, tag="fcb17", name="fcb17")
                nc.vector.tensor_scalar(out=fcb17[:], in0=fcb_sb[:], scalar1=1.702,
                                        scalar2=None,
                                        op0=mybir.AluOpType.mult)
            for m in range(24):
                ps = psM.tile([128, TS], F32, tag="m", name="m")
                for k in range(6):
                    nc.tensor.matmul(ps[:], fcw_sb[k][:, m * 128:(m + 1) * 128],
                                     h2T[k][:], start=(k == 0), stop=(k == 5))
                if SIM_GELU:
                    sgm = mTp.tile([128, TS], F32, tag="sgm", name="sgm")
                    nc.scalar.activation(out=sgm[:], in_=ps[:],
                                         func=mybir.ActivationFunctionType.Sigmoid,
                                         scale=1.702, bias=fcb17[:, m:m + 1])
                    nc.vector.tensor_scalar_add(out=mT[m][:], in0=ps[:],
                                                scalar1=fcb_sb[:, m:m + 1])
                    nc.vector.tensor_mul(out=mT[m][:], in0=mT[m][:], in1=sgm[:])
                else:
                    nc.scalar.activation(out=mT[m][:], in_=ps[:],
                                         func=mybir.ActivationFunctionType.Gelu_apprx_tanh,
                                         bias=fcb_sb[:, m:m + 1])

            # ---- MLP pr + residual --------------------------------------
            prb_bc = bcast_row(lnrow, prb[layer], 128, F16, E, tag="prb")
            ps2 = [psP.tile([128, E], F32, tag=f"p{t}", name=f"p{t}") for t in range(2)]
            for m in range(24):
                for t in range(2):
                    for n0, n1 in ((0, 512), (512, 768)):
                        nc.tensor.matmul(ps2[t][:, n0:n1],
                                         mT[m][:, t * 128:(t + 1) * 128],
                                         prw_sb[m][:, n0:n1],
                                         start=(m == 0), stop=(m == 23))
            for t in range(2):
                nc.vector.tensor_add(out=x_sb[t][:], in0=x_sb[t][:], in1=ps2[t][:])
                nc.vector.tensor_add(out=x_sb[t][:], in0=x_sb[t][:], in1=prb_bc[:])
            es_d.close()

        # ---- final LN + AllGather(all 8) + lm_head -----------------------
        lnfg_bc = bcast_row(lnrow, lnfg[0], 128, F16, E, tag="lnfg")
        lnfb_bc = bcast_row(lnrow, lnfb[0], 128, F16, E, tag="lnfb")
        es_tf = ExitStack()
        psTf = es_tf.enter_context(tc.tile_pool(name="psTf", bufs=4, space="PSUM"))
        xfT = [hTp.tile([128, TS], F16, tag=f"hT{k}", name=f"xfT{k}") for k in range(6)]
        for t in range(2):
            h_t = hpool.tile([128, E], F16, tag="h", name="h")
            layernorm_t(x_sb[t][:], lnfg_bc, lnfb_bc, h_t)
            for k in range(6):
                pt = psTf.tile([128, 128], F16, tag="trf", name="trf")
                nc.tensor.transpose(pt[:], h_t[:, k * 128:(k + 1) * 128], ident[:])
                dst = xfT[k][:, t * 128:(t + 1) * 128]
                if k % 2 == 0:
                    nc.vector.tensor_copy(dst, pt[:])
                else:
                    nc.scalar.activation(out=dst, in_=pt[:],
                                         func=mybir.ActivationFunctionType.Copy)
        es_tf.close()
        for half in range(2):
            for k in range(3):
                nc.sync.dma_start(out=xf_in[half][k * 128:(k + 1) * 128, :],
                                  in_=xfT[half * 3 + k][:])
            nc.gpsimd.collective_compute(
                "AllGather", mybir.AluOpType.bypass,
                replica_groups=g_all,
                ins=[xf_in[half].opt()],
                outs=[xf_ag[half].opt()],
            )
        es_l.close()
        es_h = es.enter_context(ExitStack())
        xfp = es_h.enter_context(tc.tile_pool(name="xfp", bufs=1))
        wtep = es_h.enter_context(tc.tile_pool(name="wtep", bufs=2))
        lop = es_h.enter_context(tc.tile_pool(name="lop", bufs=4))
        psL = es_h.enter_context(tc.tile_pool(name="psL", bufs=1, space="PSUM"))

        # xfT_full 6 x [128, 2048] fp16 (3D-AP load across 8 ranks)
        xf_full = [xfp.tile([128, B * T], F16, tag=f"xf{k}", name=f"xf{k}") for k in range(6)]
        for k in range(6):
            half, kk = divmod(k, 3)
            src_t = xf_ag[half]
            in_ap = bass.AP(
                tensor=src_t.tensor,
                offset=src_t[kk * 128, 0].offset,
                ap=[[TS, 128], [3 * 128 * TS, NC], [1, TS]],
            )
            eng = (nc.sync, nc.scalar, nc.gpsimd)[k % 3]
            eng.dma_start(out=xf_full[k][:].rearrange("p (r t) -> p r t", r=NC),
                          in_=in_ap)

        nch = (VS + 511) // 512
        for n in range(nch):
            n0 = n * 512
            nw = min(512, VS - n0)
            wte_sb = [wtep.tile([128, 512], F16, tag=f"wte{k}", name=f"wte{k}") for k in range(6)]
            for k in range(6):
                nc.gpsimd.dma_start(out=wte_sb[k][:, 0:nw],
                                    in_=wteT[n, k, :, 0:nw])
            for th in range(2):
                pss = [psL.tile([128, 512], F32, tag=f"l{t}", name=f"l{t}") for t in range(8)]
                for k in range(6):
                    for t in range(8):
                        nc.tensor.matmul(pss[t][:, 0:nw],
                                         xf_full[k][:, (th * 8 + t) * 128:(th * 8 + t + 1) * 128],
                                         wte_sb[k][:, 0:nw],
                                         start=(k == 0), stop=(k == 5))
                for t in range(8):
                    lo = lop.tile([128, 512], F16, tag="lo", name="lo")
                    if t % 2 == 0:
                        nc.vector.tensor_copy(lo[:, 0:nw], pss[t][:, 0:nw])
                    else:
                        nc.scalar.activation(out=lo[:, 0:nw], in_=pss[t][:, 0:nw],
                                             func=mybir.ActivationFunctionType.Copy)
                    eng = nc.sync if t % 2 == 0 else nc.scalar
                    row0 = (n * 16 + th * 8 + t) * 128
                    eng.dma_start(out=logits[row0:row0 + 128, 0:nw],
                                  in_=lo[:, 0:nw])

    nc.compile()
    return nc


def _block_wte(wt, nch, vs_pad):
    # [768, VS] -> [nch, 6, 128, 512] fp16 blocked
    pad = np.zeros((E, vs_pad - wt.shape[1]), np.float32)
    wtp = np.concatenate([wt, pad], axis=1)
    return np.ascontiguousarray(
        wtp.reshape(6, 128, nch, 512).transpose(2, 0, 1, 3).astype(np.float16))


def _prep_inputs(idx, wte, wpe, ln1_w, ln1_b, attn_w, attn_b, atp_w, atp_b,
                 ln2_w, ln2_b, fc_w, fc_b, pr_w, pr_b, lnf_w, lnf_b):
    idx = np.asarray(idx)
    f = lambda a: np.ascontiguousarray(np.asarray(a), dtype=np.float32)
    h = lambda a: np.ascontiguousarray(np.asarray(a), dtype=np.float16)
    wte, wpe = f(wte), f(wpe)
    x0 = wte[idx.reshape(-1)] + np.tile(wpe[:T], (B, 1))  # [2048, 768]
    wte_pad = np.zeros((VPAD, E), np.float32)
    wte_pad[:V] = wte
    wteT_full = np.ascontiguousarray(wte_pad.T)  # [768, VPAD]
    nch = (VS + 511) // 512
    vs_pad = nch * 512

    attn_w, attn_b = f(attn_w), f(attn_b)
    atp_w, atp_b = f(atp_w), f(atp_b)
    fc_w, fc_b, pr_w, pr_b = f(fc_w), f(fc_b), f(pr_w), f(pr_b)

    # fold v-bias through atp: y_true = y/den + bv  ->  + bv @ atp_w
    bv_full = attn_b[:, 2 * E:]                       # [L, 768]
    atpb_eff = atp_b + np.einsum('le,leo->lo', bv_full, atp_w)

    in_maps = []
    for c in range(NC):
        hs = 3 * (c % 4)
        q = [attn_w[:, :, (hs + hh) * HD:(hs + hh + 1) * HD] for hh in range(3)]
        k = [attn_w[:, :, E + (hs + hh) * HD:E + (hs + hh + 1) * HD] for hh in range(3)]
        v = [attn_w[:, :, 2 * E + (hs + hh) * HD:2 * E + (hs + hh + 1) * HD] for hh in range(3)]
        pad = np.zeros((L, E, HD), np.float32)
        # cols: [q0|q1, k0|k1, pad|q2, pad|k2]
        wqk_c = np.concatenate([q[0], q[1], k[0], k[1], pad, q[2], pad, k[2]], axis=2)
        qb = [attn_b[:, (hs + hh) * HD:(hs + hh + 1) * HD] for hh in range(3)]
        kb = [attn_b[:, E + (hs + hh) * HD:E + (hs + hh + 1) * HD] for hh in range(3)]
        zb = np.zeros((L, HD), np.float32)
        bqk_c = np.stack([
            np.concatenate([qb[0], qb[1]], axis=1),
            np.concatenate([kb[0], kb[1]], axis=1),
            np.concatenate([zb, qb[2]], axis=1),
            np.concatenate([zb, kb[2]], axis=1),
        ], axis=2)  # [L, 128, 4]
        wv_c = np.concatenate(v, axis=2)
        in_maps.append({
            "x0s": np.ascontiguousarray(x0[c * TS:(c + 1) * TS]),
            "wqk": h(wqk_c), "bqk": np.ascontiguousarray(bqk_c),
            "wv": h(wv_c),
            "watp": h(atp_w),
            "atpb": np.ascontiguousarray(atpb_eff),
            "fcw": h(fc_w), "fcb": np.ascontiguousarray(
                fc_b.reshape(L, 24, 128).transpose(0, 2, 1)),
            "prw": h(pr_w), "prb": pr_b,
            "ln1g": f(ln1_w), "ln1b": f(ln1_b),
            "ln2g": f(ln2_w), "ln2b": f(ln2_b),
            "lnfg": f(lnf_w).reshape(1, E), "lnfb": f(lnf_b).reshape(1, E),
            "wteT": _block_wte(wteT_full[:, c * VS:(c + 1) * VS], nch, vs_pad),
        })
    return in_maps


def kernel(trace=False, **inputs):
    if "nc" not in _CACHE:
        _CACHE["nc"] = _build_program()
    nc = _CACHE["nc"]
    in_maps = _prep_inputs(**inputs)
    res = run_bass_kernel_spmd(nc, in_maps, core_ids=list(range(NC)), trace=trace)
    _CACHE["last_result"] = res
    nch = (VS + 511) // 512
    full = np.empty((B * T, V), np.float32)
    for c in range(NC):
        blk = res.results[c]["logits"].reshape(nch, 16 * 128, 512)
        for n in range(nch):
            n0 = c * VS + n * 512
            nw = min(512, VS - n * 512)
            lo = blk[n][:, :nw]
            v0 = min(n0, V)
            v1 = min(n0 + nw, V)
            if v1 > v0:
                full[:, v0:v1] = lo[:, :v1 - v0]
    return full.reshape(B, T, V)


# revision 39
# speedup vs baseline: 1.0540x; 1.0063x over previous
"""GPT (4-layer, E=768, H=12, T=1024, B=2, V=50257) forward on 8 trn2 cores.

Sharding:
  - Residual stream x token-sharded fp32: core c owns tokens [c*256,(c+1)*256)
    of the flattened [2048] (batch-major): cores 0-3 = batch 0, 4-7 = batch 1.
  - Attention head-sharded within each batch group of 4 cores (3 heads each):
    AllGather hidden (fp16, split in 2 halves for overlap), compute q/k/v +
    scores + y for my heads over all 1024 tokens, then AllToAll the normalized
    per-head y back to token owners (uniform SPMD), out-proj token-local with
    full 768 contraction (fp32 psum, no low-precision reduction anywhere).
  - MLP fully token-local: fc weight-stationary (out [hid,tok]),
    pr activation-stationary (out [tok,E]) - no transposes inside MLP.
  - lm_head vocab-sharded fp16: AllGather lnf(x) (all 8), each core computes
    [2048, 6284] logit slice, vocab-group-outer loop with resident xfT.
  - All matmul inputs fp16 (fp32 psum accumulate); scores 2-head row-packed
    (tile_position concurrency); h2 on row-group 64-127.
"""

import sys
from contextlib import ExitStack
import numpy as np

sys.path.insert(0, "/opt/trn_rl_repo")

import concourse.bass as bass
import concourse.mybir as mybir
import concourse.tile as tile
from concourse import bacc
from concourse.bass_utils import run_bass_kernel_spmd
from concourse.masks import make_identity

L, H, E, T, V = 4, 12, 768, 1024, 50257
B = 2
NC = 8
TS = (B * T) // NC          # 256 tokens per core
VS = 6284                   # vocab slice per core (padded V = 50272)
VPAD = VS * NC
HD = 64
EPS = 1e-5
SCALE = float(1.0 / np.sqrt(np.float32(E)))
F32 = mybir.dt.float32
F16 = mybir.dt.float16

L_RUN = L  # layers actually executed (tests may truncate)
SIM_GELU = False  # sim lacks Gelu_apprx_tanh; use x*sigmoid(1.702x) for debug
DEBUG_DUMP = False  # dump layer-0 intermediates to a "dbg" output
_CACHE = {}


def _build_program():
    nc = bacc.Bacc("TRN2", target_bir_lowering=False, debug=False, num_devices=NC)

    # ---- I/O -------------------------------------------------------------
    x0s = nc.dram_tensor("x0s", [TS, E], F32, kind="ExternalInput")
    # wqk cols: [q0|q1 (128), k0|k1 (128), pad|q2 (128), pad|k2 (128)]
    wqk = nc.dram_tensor("wqk", [L, E, 512], F16, kind="ExternalInput")
    bqk = nc.dram_tensor("bqk", [L, 128, 4], F32, kind="ExternalInput")
    wv = nc.dram_tensor("wv", [L, E, 3 * HD], F16, kind="ExternalInput")
    watp = nc.dram_tensor("watp", [L, E, E], F16, kind="ExternalInput")  # full (head-major rows)
    atpb = nc.dram_tensor("atpb", [L, E], F32, kind="ExternalInput")     # includes bv@watp fold
    fcw = nc.dram_tensor("fcw", [L, E, 4 * E], F16, kind="ExternalInput")
    fcb = nc.dram_tensor("fcb", [L, 128, 24], F32, kind="ExternalInput")
    prw = nc.dram_tensor("prw", [L, 4 * E, E], F16, kind="ExternalInput")
    prb = nc.dram_tensor("prb", [L, E], F32, kind="ExternalInput")
    ln1g = nc.dram_tensor("ln1g", [L, E], F32, kind="ExternalInput")
    ln1b = nc.dram_tensor("ln1b", [L, E], F32, kind="ExternalInput")
    ln2g = nc.dram_tensor("ln2g", [L, E], F32, kind="ExternalInput")
    ln2b = nc.dram_tensor("ln2b", [L, E], F32, kind="ExternalInput")
    lnfg = nc.dram_tensor("lnfg", [1, E], F32, kind="ExternalInput")
    lnfb = nc.dram_tensor("lnfb", [1, E], F32, kind="ExternalInput")
    wteT = nc.dram_tensor("wteT", [(VS + 511) // 512, 6, 128, 512], F16, kind="ExternalInput")
    NCH = (VS + 511) // 512
    logits = nc.dram_tensor("logits", [NCH * 16 * 128, 512], F16, kind="ExternalOutput")
    dbg = (nc.dram_tensor("dbg", [12, 128, T], F16, kind="ExternalOutput")
           if DEBUG_DUMP else None)

    tri_np = (np.arange(128)[None, :] >= np.arange(128)[:, None]).astype(np.float16)
    tri = nc.inline_tensor(tri_np, name="tri_const")

    g_all = [list(range(NC))]
    g_batch = [[0, 1, 2, 3], [4, 5, 6, 7]]

    def bcast_row(pool, src_ap, n, dtype, w, tag=None):
        """Replicate a [w] DRAM row across n partitions via broadcast DMA."""
        t = pool.tile([n, w], dtype, tag=tag)
        in_ap = bass.AP(
            tensor=src_ap.tensor,
            offset=src_ap.offset,
            ap=[[0, n]] + [list(p) for p in src_ap.ap],
        )
        eng = nc.gpsimd if dtype != src_ap.dtype else nc.sync
        eng.dma_start(out=t[:], in_=in_ap)
        return t

    with tile.TileContext(nc) as tc, ExitStack() as es:
        const = es.enter_context(tc.tile_pool(name="const", bufs=1))
        xp = es.enter_context(tc.tile_pool(name="xp", bufs=1))
        lnrow = es.enter_context(tc.tile_pool(name="lnrow", bufs=1))
        stat = es.enter_context(tc.tile_pool(name="stat", bufs=2))
        hpool = es.enter_context(tc.tile_pool(name="hpool", bufs=2))
        dram = es.enter_context(tc.tile_pool(name="dram", bufs=1, space="DRAM"))

        ident_f = const.tile([128, 128], F32, name="ident_f")
        make_identity(nc, ident_f)
        ident = const.tile([128, 128], F16, name="ident")
        nc.vector.tensor_copy(ident[:], ident_f[:])
        tri_sb = const.tile([128, 128], F16, name="tri_sb")
        nc.sync.dma_start(out=tri_sb[:], in_=tri[:, :])
        eps_sb = const.tile([128, 1], F32, name="eps_sb")
        nc.vector.memset(eps_sb, EPS)
        ones3 = const.tile([128, 3], F16, name="ones3")
        nc.vector.memset(ones3, 1.0)

        # persistent residual stream [256, 768] fp32 as two [128, 768] tiles
        x_sb = [xp.tile([128, E], F32, tag=f"x{t}", name=f"x{t}") for t in range(2)]
        for t in range(2):
            nc.sync.dma_start(out=x_sb[t][:], in_=x0s[t * 128:(t + 1) * 128, :])

        # DRAM bounce buffers for collectives (fp16)
        hT_in = [dram.tile([E, 128], F16, name=f"hT_in{i}") for i in range(2)]
        hT_ag = [dram.tile([4 * E, 128], F16, name=f"hT_ag{i}") for i in range(2)]
        y_in_y = dram.tile([3 * HD, T], F16, name="y_in_y")
        y_ag = dram.tile([4 * 3 * HD, T], F16, name="y_ag")
        xf_in = [dram.tile([E, 128], F16, name=f"xf_in{i}") for i in range(2)]
        xf_ag = [dram.tile([NC * E, 128], F16, name=f"xf_ag{i}") for i in range(2)]

        def layernorm_t(x_ap, g_bc, b_bc, out_tile):
            """LN over free dim (768) of [128, 768] fp32 tile -> out fp16."""
            stats = stat.tile([128, 3, 6], F32, tag="bn_stats", name="bn_stats_t")
            xr = x_ap.rearrange("p (s d) -> p s d", s=3)
            for s in range(3):
                nc.vector.bn_stats(out=stats[:, s, :], in_=xr[:, s, :])
            mv = stat.tile([128, 2], F32, tag="bn_aggr", name="bn_aggr_t")
            nc.vector.bn_aggr(out=mv[:], in_=stats[:])
            rstd = stat.tile([128, 1], F32, tag="rstd", name="rstd_t")
            nc.scalar.activation(out=rstd[:], in_=mv[:, 1:2],
                                 func=mybir.ActivationFunctionType.Sqrt,
                                 bias=eps_sb[:], scale=1.0)
            nc.vector.reciprocal(out=rstd[:], in_=rstd[:])
            tmp = stat.tile([128, E], F32, tag="ln_tmp", name="ln_tmp")
            nc.vector.tensor_scalar(out=tmp[:], in0=x_ap,
                                    scalar1=mv[:, 0:1], scalar2=rstd[:],
                                    op0=mybir.AluOpType.subtract,
                                    op1=mybir.AluOpType.mult)
            nc.vector.tensor_mul(out=tmp[:], in0=tmp[:], in1=g_bc[:])
            nc.vector.tensor_add(out=out_tile[:], in0=tmp[:], in1=b_bc[:])

        # ---- persistent layer pools (tags reused across layers) ----------
        es_l = es.enter_context(ExitStack())
        wqkp = es_l.enter_context(tc.tile_pool(name="wqkp", bufs=1))
        wvp = es_l.enter_context(tc.tile_pool(name="wvp", bufs=1))
        watpp = es_l.enter_context(tc.tile_pool(name="watpp", bufs=1))
        fcwp = es_l.enter_context(tc.tile_pool(name="fcwp", bufs=1))
        prwp = es_l.enter_context(tc.tile_pool(name="prwp", bufs=1))
        bias_p = es_l.enter_context(tc.tile_pool(name="bias_p", bufs=2))
        hTp = es_l.enter_context(tc.tile_pool(name="hTp", bufs=1))
        hTbp = es_l.enter_context(tc.tile_pool(name="hTbp", bufs=1))
        qkp = es_l.enter_context(tc.tile_pool(name="qkp", bufs=1))
        vp = es_l.enter_context(tc.tile_pool(name="vp", bufs=1))
        ep = es_l.enter_context(tc.tile_pool(name="ep", bufs=2))
        yp = es_l.enter_context(tc.tile_pool(name="yp", bufs=1))
        sm = es_l.enter_context(tc.tile_pool(name="sm", bufs=2))
        mTp = es_l.enter_context(tc.tile_pool(name="mTp", bufs=1))
        yallp = es_l.enter_context(tc.tile_pool(name="yallp", bufs=1))

        # v_sb layout [128, 258]: h0 [v0|1] at 0:65, h1 [1|pad63|v1] at
        # 65:193 (den at psum row 0, y1 at rows 64-127 for packed
        # out-proj), h2 [v2|1] at 193:258. Constant cols written once.
        v_sb = [vp.tile([128, 258], F16, tag=f"v{t}", name=f"v{t}") for t in range(8)]
        for t in range(8):
            nc.vector.tensor_copy(v_sb[t][:, 64:65], ones3[:, 0:1])
            nc.vector.tensor_copy(v_sb[t][:, 65:66], ones3[:, 1:2])
            nc.vector.tensor_copy(v_sb[t][:, 257:258], ones3[:, 2:3])
            nc.vector.memset(v_sb[t][:, 66:129], 0.0)

        def emit_stage(t, g_bc, b_bc, stage_tiles, tin, tag_out, groups):
            """LN(x_sb[t]) -> transpose -> store token-half -> AG trigger."""
            h_t = hpool.tile([128, E], F16, tag="h", name="h")
            layernorm_t(x_sb[t][:], g_bc, b_bc, h_t)
            es_ts = ExitStack()
            psT = es_ts.enter_context(tc.tile_pool(name="psT", bufs=4, space="PSUM"))
            for k in range(6):
                pt = psT.tile([128, 128], F16, tag="tr", name="tr")
                nc.tensor.transpose(pt[:], h_t[:, k * 128:(k + 1) * 128], ident[:])
                dst = stage_tiles[k][:, t * 128:(t + 1) * 128]
                if k % 2 == 0:
                    nc.vector.tensor_copy(dst, pt[:])
                else:
                    nc.scalar.activation(out=dst, in_=pt[:],
                                         func=mybir.ActivationFunctionType.Copy)
            es_ts.close()
            for k in range(6):
                nc.sync.dma_start(out=tin[t][k * 128:(k + 1) * 128, :],
                                  in_=stage_tiles[k][:, t * 128:(t + 1) * 128])
            nc.gpsimd.collective_compute(
                "AllGather", mybir.AluOpType.bypass,
                replica_groups=groups,
                ins=[tin[t].opt()],
                outs=[tag_out[t].opt()],
            )

        def stage_params(i):
            if i < L_RUN:
                g = bcast_row(lnrow, ln1g[i], 128, F16, E, tag="ln1g")
                b = bcast_row(lnrow, ln1b[i], 128, F16, E, tag="ln1b")
                tiles = [hTp.tile([128, TS], F16, tag=f"hT{k}", name=f"hT{k}")
                         for k in range(6)]
                return g, b, tiles, hT_in, hT_ag, g_batch
            g = bcast_row(lnrow, lnfg[0], 128, F16, E, tag="ln1g")
            b = bcast_row(lnrow, lnfb[0], 128, F16, E, tag="ln1b")
            tiles = [hTp.tile([128, TS], F16, tag=f"hT{k}", name=f"xfT{k}")
                     for k in range(6)]
            return g, b, tiles, xf_in, xf_ag, g_all

        # prologue: stage 0 (both token halves, no overlap available)
        sp = stage_params(0)
        for t in range(2):
            emit_stage(t, sp[0], sp[1], sp[2], sp[3], sp[4], sp[5])

        for layer in range(L_RUN):
            # ---- weight loads (gpsimd queue; Tile schedules early) ------
            wqk_sb = [wqkp.tile([128, 512], F16, tag=f"wqk{k}", name=f"wqk{k}") for k in range(6)]
            wv_sb = [wvp.tile([128, 192], F16, tag=f"wv{k}", name=f"wv{k}") for k in range(6)]
            bqk_sb = bias_p.tile([128, 4], F32, tag="bqk", name="bqk")
            nc.sync.dma_start(out=bqk_sb[:], in_=bqk[layer])
            fcb_sb = bias_p.tile([128, 24], F32, tag="fcb", name="fcb")
            nc.sync.dma_start(out=fcb_sb[:], in_=fcb[layer])

            # load hTb 6 x [128, 1024] fp16 (2 token-half DMAs per k-chunk)
            hTb = [hTbp.tile([128, T], F16, tag=f"hTb{k}", name=f"hTb{k}") for k in range(6)]
            for k in range(6):
                for th in range(2):
                    src = hT_ag[th]
                    in_ap = bass.AP(
                        tensor=src.tensor,
                        offset=src[k * 128, 0].offset,
                        ap=[[128, 128], [E * 128, 4], [1, 128]],
                    )
                    eng = (nc.sync, nc.scalar)[(2 * k + th) % 2]
                    eng.dma_start(
                        out=hTb[k][:].rearrange("p (r u tt) -> p r u tt",
                                                r=4, u=2)[:, :, th, :],
                        in_=in_ap)

            if DEBUG_DUMP and layer == 0:
                nc.sync.dma_start(out=dbg[0], in_=hTb[0][:])
                nc.sync.dma_start(out=dbg[11][:, 0:128], in_=tri_sb[:])

            # weight loads on gpsimd AFTER collective triggers (in-order queue)
            watp_sb = [watpp.tile([128, E], F16, tag=f"wa{k}", name=f"wa{k}") for k in range(6)]
            for k in range(6):
                nc.gpsimd.dma_start(out=wqk_sb[k][:], in_=wqk[layer, k * 128:(k + 1) * 128, :])
                nc.gpsimd.dma_start(out=wv_sb[k][:], in_=wv[layer, k * 128:(k + 1) * 128, :])
            for k in range(6):
                nc.gpsimd.dma_start(out=watp_sb[k][:], in_=watp[layer, k * 128:(k + 1) * 128, :])
            fcw_sb = [fcwp.tile([128, 4 * E], F16, tag=f"fcw{k}", name=f"fcw{k}") for k in range(6)]
            for k in range(6):
                nc.gpsimd.dma_start(out=fcw_sb[k][:], in_=fcw[layer, k * 128:(k + 1) * 128, :])
            prw_sb = [prwp.tile([128, E], F16, tag=f"prw{m}", name=f"prw{m}") for m in range(24)]
            for m in range(24):
                nc.gpsimd.dma_start(out=prw_sb[m][:], in_=prw[layer, m * 128:(m + 1) * 128, :])

            # ---- QKV ----------------------------------------------------
            es_a = ExitStack()
            psQK = es_a.enter_context(tc.tile_pool(name="psQK", bufs=3, space="PSUM"))
            psV = es_a.enter_context(tc.tile_pool(name="psV", bufs=2, space="PSUM"))
            # qT2/kT2: rows 0-63 head0, 64-127 head1; q1T/k1T rows 64-127 head2
            qT2 = qkp.tile([128, T], F16, tag="qT2", name="qT2")
            kT2 = qkp.tile([128, T], F16, tag="kT2", name="kT2")
            q1T = qkp.tile([128, T], F16, tag="q1T", name="q1T")
            k1T = qkp.tile([128, T], F16, tag="k1T", name="k1T")
            qk_dsts = [qT2, kT2, q1T, k1T]
            for s in range(4):
                ps = psQK.tile([128, T], F32, tag="qk", name="qk")
                for n in range(2):
                    for k in range(6):
                        nc.tensor.matmul(ps[:, n * 512:(n + 1) * 512],
                                         wqk_sb[k][:, s * 128:(s + 1) * 128],
                                         hTb[k][:, n * 512:(n + 1) * 512],
                                         start=(k == 0), stop=(k == 5))
                if s < 2:
                    nc.vector.tensor_scalar_add(out=qk_dsts[s][:], in0=ps[:],
                                                scalar1=bqk_sb[:, s:s + 1])
                else:
                    nc.vector.tensor_scalar_add(out=qk_dsts[s][64:128, :],
                                                in0=ps[64:128, :],
                                                scalar1=bqk_sb[64:128, s:s + 1])
            for t in range(8):
                ps = psV.tile([128, 192], F32, tag="v", name="v")
                for k in range(6):
                    nc.tensor.matmul(ps[:], hTb[k][:, t * 128:(t + 1) * 128],
                                     wv_sb[k][:], start=(k == 0), stop=(k == 5))
                if t % 2 == 0:
                    nc.vector.tensor_copy(v_sb[t][:, 0:64], ps[:, 0:64])
                    nc.vector.tensor_copy(v_sb[t][:, 129:193], ps[:, 64:128])
                    nc.vector.tensor_copy(v_sb[t][:, 193:257], ps[:, 128:192])
                else:
                    nc.scalar.activation(out=v_sb[t][:, 0:64], in_=ps[:, 0:64],
                                         func=mybir.ActivationFunctionType.Copy)
                    nc.scalar.activation(out=v_sb[t][:, 129:193], in_=ps[:, 64:128],
                                         func=mybir.ActivationFunctionType.Copy)
                    nc.scalar.activation(out=v_sb[t][:, 193:257], in_=ps[:, 128:192],
                                         func=mybir.ActivationFunctionType.Copy)

            if DEBUG_DUMP and layer == 0:
                nc.sync.dma_start(out=dbg[1], in_=qT2[:])
                nc.sync.dma_start(out=dbg[2], in_=kT2[:])
                nc.sync.dma_start(out=dbg[3], in_=k1T[:])
                nc.sync.dma_start(out=dbg[10][:, 0:258], in_=v_sb[0][:])

            # ---- attention: scores + y, head 0/1 row-packed --------------
            es_a.close()
            es_b = ExitStack()
            psS = es_b.enter_context(tc.tile_pool(name="psS", bufs=2, space="PSUM"))
            psY = es_b.enter_context(tc.tile_pool(name="psY", bufs=1, space="PSUM"))
            # h0: y rows 0:64, den 64; h1: den 0, y 64:128; h2: y 0:64, den 64
            yps0 = psY.tile([65, T], F32, tag="y0", name="y0")
            yps1 = psY.tile([128, T], F32, tag="y1", name="y1")
            yps2 = psY.tile([65, T], F32, tag="y2", name="y2")
            heads = [(qT2[0:64, :], kT2[0:64, :], yps0, 0, 65),
                     (qT2[64:128, :], kT2[64:128, :], yps1, 65, 193),
                     (q1T[64:128, :], k1T[64:128, :], yps2, 193, 258)]
            for j in range(8):
                qs = j * 128
                qlen = T - qs
                for h in range(3):
                    qT_h, kT_h, yout, v0, v1 = heads[h]
                    e_sb = ep.tile([128, T], F16, tag=f"e{h}", name=f"e{h}")
                    off = 0
                    while off < qlen:
                        cl = min(512, qlen - off)
                        pss = psS.tile([128, 512], F32, tag="s", name="s")
                        nc.tensor.matmul(pss[:, 0:cl], kT_h[:, qs:qs + 128],
                                         qT_h[:, qs + off: qs + off + cl],
                                         start=True, stop=True)
                        nc.scalar.activation(out=e_sb[:, off:off + cl],
                                             in_=pss[:, 0:cl],
                                             func=mybir.ActivationFunctionType.Exp,
                                             scale=SCALE)
                        off += cl
                    nc.vector.tensor_mul(out=e_sb[:, 0:128], in0=e_sb[:, 0:128],
                                         in1=tri_sb[:])
                    if qs < 512:
                        nc.tensor.matmul(yout[:, qs:512],
                                         v_sb[j][:, v0:v1],
                                         e_sb[:, 0:512 - qs],
                                         start=(j == 0), stop=(j == 3))
                    nc.tensor.matmul(yout[:, max(qs, 512):T],
                                     v_sb[j][:, v0:v1],
                                     e_sb[:, max(qs, 512) - qs:qlen],
                                     start=(j == 0), stop=(j == 7))

            # ---- normalize into packed y tiles ---------------------------
            # y01T rows 0:64 = head0, 64:128 = head1; y2T rows 0:64 = head2
            y01T = yp.tile([128, T], F16, tag="y01T", name="y01T")
            y2T = yp.tile([64, T], F16, tag="y2T", name="y2T")
            for h, (yout, den_row, dst) in enumerate(
                    [(yps0, 64, y01T[0:64, :]), (yps1, 0, y01T[64:128, :]),
                     (yps2, 64, y2T[:])]):
                # den psum->sbuf (gpsimd cannot read PSUM), broadcast to all
                # 128 partitions, then one DVE divide
                den_sb = sm.tile([1, T], F32, tag="den", name="den")
                nc.vector.tensor_copy(den_sb[:], yout[den_row:den_row + 1, :])
                bcf = sm.tile([128, T], F32, tag="bcf", name="bcf")
                nc.gpsimd.partition_broadcast(bcf[:], den_sb[:])
                lo = 64 if h == 1 else 0
                ysrc = yout[64:128, :] if h == 1 else yout[0:64, :]
                nc.vector.reciprocal_approx_fast(out=bcf[:], in_=bcf[:])
                nc.vector.tensor_mul(out=dst, in0=ysrc, in1=bcf[lo:lo + 64, :])
            if DEBUG_DUMP and layer == 0:
                nc.sync.dma_start(out=dbg[4], in_=y01T[:])
                nc.sync.dma_start(out=dbg[5][0:64, :], in_=y2T[:])
            es_b.close()

            # ---- AllGather y (all heads, all tokens) ---------------------
            nc.sync.dma_start(out=y_in_y[0:128, :], in_=y01T[:])
            nc.sync.dma_start(out=y_in_y[128:192, :], in_=y2T[:])
            nc.gpsimd.collective_compute(
                "AllGather", mybir.AluOpType.bypass,
                replica_groups=g_batch,
                ins=[y_in_y.opt()],
                outs=[y_ag.opt()],
            )
            # own-token slice [768, 256] via rank-dependent column offset
            r4 = nc.gpsimd.partition_id() % 4
            yall = [yallp.tile([128, TS], F16, tag=f"ya{k}", name=f"ya{k}") for k in range(6)]
            for k in range(6):
                in_ap = bass.AP(
                    tensor=y_ag.tensor,
                    offset=r4 * TS + y_ag[k * 128, 0].offset,
                    ap=[[T, 128], [1, TS]],
                    dep_tracking_offset=y_ag[k * 128, 0].offset,
                )
                nc.gpsimd.dma_start(out=yall[k][:], in_=in_ap)

            if DEBUG_DUMP and layer == 0:
                nc.sync.dma_start(out=dbg[6][:, 0:TS], in_=yall[0][:])
                nc.sync.dma_start(out=dbg[7][:, 0:TS], in_=yall[5][:])
                yag_chk = hpool.tile([128, T], F16, tag="yagchk", name="yagchk")
                nc.sync.dma_start(out=yag_chk[:], in_=bass.AP(
                    tensor=y_ag.tensor, offset=y_ag[0, 0].offset,
                    ap=[[T, 128], [1, T]]))
                nc.sync.dma_start(out=dbg[3][:, :], in_=yag_chk[:])

            # ---- out-proj (own 256 tokens, full 768 contraction) ---------
            es_c = ExitStack()
            psO = es_c.enter_context(tc.tile_pool(name="psO", bufs=2, space="PSUM"))
            atpb_bc = bcast_row(lnrow, atpb[layer], 128, F16, E, tag="atpb")
            for t in range(2):
                po = psO.tile([128, E], F32, tag="o", name="o")
                for n0, n1 in ((0, 512), (512, 768)):
                    for k in range(6):
                        nc.tensor.matmul(po[:, n0:n1],
                                         yall[k][:, t * 128:(t + 1) * 128],
                                         watp_sb[k][:, n0:n1],
                                         start=(k == 0), stop=(k == 5))
                nc.vector.tensor_add(out=x_sb[t][:], in0=x_sb[t][:], in1=po[:])
                nc.vector.tensor_add(out=x_sb[t][:], in0=x_sb[t][:], in1=atpb_bc[:])
            if DEBUG_DUMP and layer == 0:
                for t in range(2):
                    xc = hpool.tile([128, E], F16, tag="ao", name="xc")
                    nc.vector.tensor_copy(xc[:], x_sb[t][:])
                    nc.sync.dma_start(out=dbg[8 + t][:, 0:E], in_=xc[:])
            es_c.close()

            # ---- LN2 + transpose -> h2T ---------------------------------
            ln2g_bc = bcast_row(lnrow, ln2g[layer], 128, F16, E, tag="ln2g")
            ln2b_bc = bcast_row(lnrow, ln2b[layer], 128, F16, E, tag="ln2b")
            es_t2 = ExitStack()
            psT2 = es_t2.enter_context(tc.tile_pool(name="psT2", bufs=4, space="PSUM"))
            h2T = [hTp.tile([128, TS], F16, tag=f"h2T{k}", name=f"h2T{k}") for k in range(6)]
            for t in range(2):
                h_t = hpool.tile([128, E], F16, tag="h", name="h")
                layernorm_t(x_sb[t][:], ln2g_bc, ln2b_bc, h_t)
                for k in range(6):
                    pt = psT2.tile([128, 128], F16, tag="tr2", name="tr2")
                    nc.tensor.transpose(pt[:], h_t[:, k * 128:(k + 1) * 128], ident[:])
                    dst = h2T[k][:, t * 128:(t + 1) * 128]
                    if k % 2 == 0:
                        nc.vector.tensor_copy(dst, pt[:])
                    else:
                        nc.scalar.activation(out=dst, in_=pt[:],
                                             func=mybir.ActivationFunctionType.Copy)
            es_t2.close()

            # ---- MLP fc: mT[m] = gelu(fcw[:,m].T @ h2T + fcb[m]) ---------
            es_fc = ExitStack()
            psM = es_fc.enter_context(tc.tile_pool(name="psM", bufs=4, space="PSUM"))
            mT = [mTp.tile([128, TS], F16, tag=f"mT{m}", name=f"mT{m}") for m in range(24)]
            if SIM_GELU:
                fcb17 = bias_p.tile([128, 24], F32, tag="fcb17", name="fcb17")
                nc.vector.tensor_scalar(out=fcb17[:], in0=fcb_sb[:], scalar1=1.702,
                                        scalar2=None,
                                        op0=mybir.AluOpType.mult)
            for m in range(24):
                ps = psM.tile([128, TS], F32, tag="m", name="m")
                for k in range(6):
                    nc.tensor.matmul(ps[:], fcw_sb[k][:, m * 128:(m + 1) * 128],
                                     h2T[k][:], start=(k == 0), stop=(k == 5))
                if SIM_GELU:
                    sgm = mTp.tile([128, TS], F32, tag="sgm", name="sgm")
                    nc.scalar.activation(out=sgm[:], in_=ps[:],
                                         func=mybir.ActivationFunctionType.Sigmoid,
                                         scale=1.702, bias=fcb17[:, m:m + 1])
                    nc.vector.tensor_scalar_add(out=mT[m][:], in0=ps[:],
                                                scalar1=fcb_sb[:, m:m + 1])
                    nc.vector.tensor_mul(out=mT[m][:], in0=mT[m][:], in1=sgm[:])
                else:
                    nc.scalar.activation(out=mT[m][:], in_=ps[:],
                                         func=mybir.ActivationFunctionType.Gelu_apprx_tanh,
                                         bias=fcb_sb[:, m:m + 1])

            es_fc.close()
            # ---- MLP pr + residual, t-split; emit next stage per half ----
            es_d = ExitStack()
            psP = es_d.enter_context(tc.tile_pool(name="psP", bufs=1, space="PSUM"))
            prb_bc = bcast_row(lnrow, prb[layer], 128, F16, E, tag="prb")
            spn = stage_params(layer + 1)
            for t in range(2):
                ps2 = psP.tile([128, E], F32, tag=f"p{t}", name=f"p{t}")
                for m in range(24):
                    for n0, n1 in ((0, 512), (512, 768)):
                        nc.tensor.matmul(ps2[:, n0:n1],
                                         mT[m][:, t * 128:(t + 1) * 128],
                                         prw_sb[m][:, n0:n1],
                                         start=(m == 0), stop=(m == 23))
                nc.vector.tensor_add(out=x_sb[t][:], in0=x_sb[t][:], in1=ps2[:])
                nc.vector.tensor_add(out=x_sb[t][:], in0=x_sb[t][:], in1=prb_bc[:])
                emit_stage(t, spn[0], spn[1], spn[2], spn[3], spn[4], spn[5])
            es_d.close()

        # ---- lm_head (xf stage emitted by last layer / prologue) --------
        es_l.close()
        es_h = es.enter_context(ExitStack())
        xfp = es_h.enter_context(tc.tile_pool(name="xfp", bufs=1))
        wtep = es_h.enter_context(tc.tile_pool(name="wtep", bufs=2))
        lop = es_h.enter_context(tc.tile_pool(name="lop", bufs=4))
        psL = es_h.enter_context(tc.tile_pool(name="psL", bufs=1, space="PSUM"))

        # xfT_full 6 x [128, 2048] fp16 (3D-AP load across 8 ranks)
        xf_full = [xfp.tile([128, B * T], F16, tag=f"xf{k}", name=f"xf{k}") for k in range(6)]
        for k in range(6):
            for th in range(2):
                src_t = xf_ag[th]
                in_ap = bass.AP(
                    tensor=src_t.tensor,
                    offset=src_t[k * 128, 0].offset,
                    ap=[[128, 128], [E * 128, NC], [1, 128]],
                )
                eng = (nc.sync, nc.scalar)[(2 * k + th) % 2]
                eng.dma_start(
                    out=xf_full[k][:].rearrange("p (r u tt) -> p r u tt",
                                                r=NC, u=2)[:, :, th, :],
                    in_=in_ap)

        nch = (VS + 511) // 512
        for n in range(nch):
            n0 = n * 512
            nw = min(512, VS - n0)
            wte_sb = [wtep.tile([128, 512], F16, tag=f"wte{k}", name=f"wte{k}") for k in range(6)]
            for k in range(6):
                nc.gpsimd.dma_start(out=wte_sb[k][:, 0:nw],
                                    in_=wteT[n, k, :, 0:nw])
            for th in range(2):
                pss = [psL.tile([128, 512], F32, tag=f"l{t}", name=f"l{t}") for t in range(8)]
                for k in range(6):
                    for t in range(8):
                        nc.tensor.matmul(pss[t][:, 0:nw],
                                         xf_full[k][:, (th * 8 + t) * 128:(th * 8 + t + 1) * 128],
                                         wte_sb[k][:, 0:nw],
                                         start=(k == 0), stop=(k == 5))
                for t in range(8):
                    lo = lop.tile([128, 512], F16, tag="lo", name="lo")
                    if t % 2 == 0:
                        nc.vector.tensor_copy(lo[:, 0:nw], pss[t][:, 0:nw])
                    else:
                        nc.scalar.activation(out=lo[:, 0:nw], in_=pss[t][:, 0:nw],
                                             func=mybir.ActivationFunctionType.Copy)
                    eng = nc.sync if t % 2 == 0 else nc.scalar
                    row0 = (n * 16 + th * 8 + t) * 128
                    eng.dma_start(out=logits[row0:row0 + 128, 0:nw],
                                  in_=lo[:, 0:nw])

    nc.compile()
    return nc


def _block_wte(wt, nch, vs_pad):
    # [768, VS] -> [nch, 6, 128, 512] fp16 blocked
    pad = np.zeros((E, vs_pad - wt.shape[1]), np.float32)
    wtp = np.concatenate([wt, pad], axis=1)
    return np.ascontiguousarray(
        wtp.reshape(6, 128, nch, 512).transpose(2, 0, 1, 3).astype(np.float16))


def _prep_inputs(idx, wte, wpe, ln1_w, ln1_b, attn_w, attn_b, atp_w, atp_b,
                 ln2_w, ln2_b, fc_w, fc_b, pr_w, pr_b, lnf_w, lnf_b):
    idx = np.asarray(idx)
    f = lambda a: np.ascontiguousarray(np.asarray(a), dtype=np.float32)
    h = lambda a: np.ascontiguousarray(np.asarray(a), dtype=np.float16)
    wte, wpe = f(wte), f(wpe)
    x0 = wte[idx.reshape(-1)] + np.tile(wpe[:T], (B, 1))  # [2048, 768]
    wte_pad = np.zeros((VPAD, E), np.float32)
    wte_pad[:V] = wte
    wteT_full = np.ascontiguousarray(wte_pad.T)  # [768, VPAD]
    nch = (VS + 511) // 512
    vs_pad = nch * 512

    attn_w, attn_b = f(attn_w), f(attn_b)
    atp_w, atp_b = f(atp_w), f(atp_b)
    fc_w, fc_b, pr_w, pr_b = f(fc_w), f(fc_b), f(pr_w), f(pr_b)

    # fold v-bias through atp: y_true = y/den + bv  ->  + bv @ atp_w
    bv_full = attn_b[:, 2 * E:]                       # [L, 768]
    atpb_eff = atp_b + np.einsum('le,leo->lo', bv_full, atp_w)

    in_maps = []
    for c in range(NC):
        hs = 3 * (c % 4)
        q = [attn_w[:, :, (hs + hh) * HD:(hs + hh + 1) * HD] for hh in range(3)]
        k = [attn_w[:, :, E + (hs + hh) * HD:E + (hs + hh + 1) * HD] for hh in range(3)]
        v = [attn_w[:, :, 2 * E + (hs + hh) * HD:2 * E + (hs + hh + 1) * HD] for hh in range(3)]
        pad = np.zeros((L, E, HD), np.float32)
        # cols: [q0|q1, k0|k1, pad|q2, pad|k2]
        wqk_c = np.concatenate([q[0], q[1], k[0], k[1], pad, q[2], pad, k[2]], axis=2)
        qb = [attn_b[:, (hs + hh) * HD:(hs + hh + 1) * HD] for hh in range(3)]
        kb = [attn_b[:, E + (hs + hh) * HD:E + (hs + hh + 1) * HD] for hh in range(3)]
        zb = np.zeros((L, HD), np.float32)
        bqk_c = np.stack([
            np.concatenate([qb[0], qb[1]], axis=1),
            np.concatenate([kb[0], kb[1]], axis=1),
            np.concatenate([zb, qb[2]], axis=1),
            np.concatenate([zb, kb[2]], axis=1),
        ], axis=2)  # [L, 128, 4]
        wv_c = np.concatenate(v, axis=2)
        in_maps.append({
            "x0s": np.ascontiguousarray(x0[c * TS:(c + 1) * TS]),
            "wqk": h(wqk_c), "bqk": np.ascontiguousarray(bqk_c),
            "wv": h(wv_c),
            "watp": h(atp_w),
            "atpb": np.ascontiguousarray(atpb_eff),
            "fcw": h(fc_w), "fcb": np.ascontiguousarray(
                fc_b.reshape(L, 24, 128).transpose(0, 2, 1)),
            "prw": h(pr_w), "prb": pr_b,
            "ln1g": f(ln1_w), "ln1b": f(ln1_b),
            "ln2g": f(ln2_w), "ln2b": f(ln2_b),
            "lnfg": f(lnf_w).reshape(1, E), "lnfb": f(lnf_b).reshape(1, E),
            "wteT": _block_wte(wteT_full[:, c * VS:(c + 1) * VS], nch, vs_pad),
        })
    return in_maps


def kernel(trace=False, **inputs):
    if "nc" not in _CACHE:
        _CACHE["nc"] = _build_program()
    nc = _CACHE["nc"]
    in_maps = _prep_inputs(**inputs)
    res = run_bass_kernel_spmd(nc, in_maps, core_ids=list(range(NC)), trace=trace)
    _CACHE["last_result"] = res
    nch = (VS + 511) // 512
    full = np.empty((B * T, V), np.float32)
    for c in range(NC):
        blk = res.results[c]["logits"].reshape(nch, 16 * 128, 512)
        for n in range(nch):
            n0 = c * VS + n * 512
            nw = min(512, VS - n * 512)
            lo = blk[n][:, :nw]
            v0 = min(n0, V)
            v1 = min(n0 + nw, V)
            if v1 > v0:
                full[:, v0:v1] = lo[:, :v1 - v0]
    return full.reshape(B, T, V)


# revision 41
# speedup vs baseline: 1.0939x; 1.0379x over previous
"""GPT (4-layer, E=768, H=12, T=1024, B=2, V=50257) forward on 8 trn2 cores.

Sharding:
  - Residual stream x token-sharded fp32: core c owns tokens [c*256,(c+1)*256)
    of the flattened [2048] (batch-major): cores 0-3 = batch 0, 4-7 = batch 1.
  - Attention head-sharded within each batch group of 4 cores (3 heads each):
    AllGather hidden (fp16, split in 2 halves for overlap), compute q/k/v +
    scores + y for my heads over all 1024 tokens, then AllToAll the normalized
    per-head y back to token owners (uniform SPMD), out-proj token-local with
    full 768 contraction (fp32 psum, no low-precision reduction anywhere).
  - MLP fully token-local: fc weight-stationary (out [hid,tok]),
    pr activation-stationary (out [tok,E]) - no transposes inside MLP.
  - lm_head vocab-sharded fp16: AllGather lnf(x) (all 8), each core computes
    [2048, 6284] logit slice, vocab-group-outer loop with resident xfT.
  - All matmul inputs fp16 (fp32 psum accumulate); scores 2-head row-packed
    (tile_position concurrency); h2 on row-group 64-127.
"""

import sys
from contextlib import ExitStack
import numpy as np

sys.path.insert(0, "/opt/trn_rl_repo")

import concourse.bass as bass
import concourse.mybir as mybir
import concourse.tile as tile
from concourse import bacc
from concourse.bass_utils import run_bass_kernel_spmd
from concourse.masks import make_identity

L, H, E, T, V = 4, 12, 768, 1024, 50257
B = 2
NC = 8
TS = (B * T) // NC          # 256 tokens per core
VS = 6284                   # vocab slice per core (padded V = 50272)
VPAD = VS * NC
HD = 64
EPS = 1e-5
SCALE = float(1.0 / np.sqrt(np.float32(E)))
F32 = mybir.dt.float32
F16 = mybir.dt.float16

L_RUN = L  # layers actually executed (tests may truncate)
SIM_GELU = False  # sim lacks Gelu_apprx_tanh; use x*sigmoid(1.702x) for debug
DEBUG_DUMP = False  # dump layer-0 intermediates to a "dbg" output
_CACHE = {}


def _build_program():
    nc = bacc.Bacc("TRN2", target_bir_lowering=False, debug=False, num_devices=NC)

    # ---- I/O -------------------------------------------------------------
    x0s = nc.dram_tensor("x0s", [TS, E], F32, kind="ExternalInput")
    # wqk cols: [q0|q1 (128), k0|k1 (128), pad|q2 (128), pad|k2 (128)]
    wqk = nc.dram_tensor("wqk", [L, E, 512], F16, kind="ExternalInput")
    bqk = nc.dram_tensor("bqk", [L, 128, 4], F32, kind="ExternalInput")
    wv = nc.dram_tensor("wv", [L, E, 3 * HD], F16, kind="ExternalInput")
    watp = nc.dram_tensor("watp", [L, E, E], F16, kind="ExternalInput")  # full (head-major rows)
    atpb = nc.dram_tensor("atpb", [L, E], F32, kind="ExternalInput")     # includes bv@watp fold
    fcw = nc.dram_tensor("fcw", [L, E, 4 * E], F16, kind="ExternalInput")
    fcb = nc.dram_tensor("fcb", [L, 128, 24], F32, kind="ExternalInput")
    prw = nc.dram_tensor("prw", [L, 4 * E, E], F16, kind="ExternalInput")
    prb = nc.dram_tensor("prb", [L, E], F32, kind="ExternalInput")
    ln1g = nc.dram_tensor("ln1g", [L, E], F32, kind="ExternalInput")
    ln1b = nc.dram_tensor("ln1b", [L, E], F32, kind="ExternalInput")
    ln2g = nc.dram_tensor("ln2g", [L, E], F32, kind="ExternalInput")
    ln2b = nc.dram_tensor("ln2b", [L, E], F32, kind="ExternalInput")
    lnfg = nc.dram_tensor("lnfg", [1, E], F32, kind="ExternalInput")
    lnfb = nc.dram_tensor("lnfb", [1, E], F32, kind="ExternalInput")
    wteT = nc.dram_tensor("wteT", [(VS + 511) // 512, 6, 128, 512], F16, kind="ExternalInput")
    NCH = (VS + 511) // 512
    logits = nc.dram_tensor("logits", [NCH * 16 * 128, 512], F16, kind="ExternalOutput")
    dbg = (nc.dram_tensor("dbg", [12, 128, T], F16, kind="ExternalOutput")
           if DEBUG_DUMP else None)

    tri_np = (np.arange(128)[None, :] >= np.arange(128)[:, None]).astype(np.float16)
    tri = nc.inline_tensor(tri_np, name="tri_const")

    g_all = [list(range(NC))]
    g_batch = [[0, 1, 2, 3], [4, 5, 6, 7]]

    def bcast_row(pool, src_ap, n, dtype, w, tag=None):
        """Replicate a [w] DRAM row across n partitions via broadcast DMA."""
        t = pool.tile([n, w], dtype, tag=tag)
        in_ap = bass.AP(
            tensor=src_ap.tensor,
            offset=src_ap.offset,
            ap=[[0, n]] + [list(p) for p in src_ap.ap],
        )
        eng = nc.gpsimd if dtype != src_ap.dtype else nc.sync
        eng.dma_start(out=t[:], in_=in_ap)
        return t

    with tile.TileContext(nc) as tc, ExitStack() as es:
        const = es.enter_context(tc.tile_pool(name="const", bufs=1))
        xp = es.enter_context(tc.tile_pool(name="xp", bufs=1))
        lnrow = es.enter_context(tc.tile_pool(name="lnrow", bufs=1))
        stat = es.enter_context(tc.tile_pool(name="stat", bufs=2))
        hpool = es.enter_context(tc.tile_pool(name="hpool", bufs=2))
        dram = es.enter_context(tc.tile_pool(name="dram", bufs=1, space="DRAM"))

        ident_f = const.tile([128, 128], F32, name="ident_f")
        make_identity(nc, ident_f)
        ident = const.tile([128, 128], F16, name="ident")
        nc.vector.tensor_copy(ident[:], ident_f[:])
        tri_sb = const.tile([128, 128], F16, name="tri_sb")
        nc.sync.dma_start(out=tri_sb[:], in_=tri[:, :])
        eps_sb = const.tile([128, 1], F32, name="eps_sb")
        nc.vector.memset(eps_sb, EPS)
        ones3 = const.tile([128, 3], F16, name="ones3")
        nc.vector.memset(ones3, 1.0)

        # persistent residual stream [256, 768] fp32 as two [128, 768] tiles
        x_sb = [xp.tile([128, E], F32, tag=f"x{t}", name=f"x{t}") for t in range(2)]
        for t in range(2):
            nc.sync.dma_start(out=x_sb[t][:], in_=x0s[t * 128:(t + 1) * 128, :])

        # DRAM bounce buffers for collectives (fp16)
        hT_in = [dram.tile([E, 128], F16, name=f"hT_in{i}") for i in range(2)]
        hT_ag = [dram.tile([4 * E, 128], F16, name=f"hT_ag{i}") for i in range(2)]
        y01_in = dram.tile([2 * HD, T], F16, name="y01_in")
        y01_ag = dram.tile([4 * 2 * HD, T], F16, name="y01_ag")
        y2_in = dram.tile([HD, T], F16, name="y2_in")
        y2_ag = dram.tile([4 * HD, T], F16, name="y2_ag")
        xf_in = [dram.tile([E, 128], F16, name=f"xf_in{i}") for i in range(2)]
        xf_ag = [dram.tile([NC * E, 128], F16, name=f"xf_ag{i}") for i in range(2)]

        def layernorm_t(x_ap, g_bc, b_bc, out_tile):
            """LN over free dim (768) of [128, 768] fp32 tile -> out fp16."""
            stats = stat.tile([128, 3, 6], F32, tag="bn_stats", name="bn_stats_t")
            xr = x_ap.rearrange("p (s d) -> p s d", s=3)
            for s in range(3):
                nc.vector.bn_stats(out=stats[:, s, :], in_=xr[:, s, :])
            mv = stat.tile([128, 2], F32, tag="bn_aggr", name="bn_aggr_t")
            nc.vector.bn_aggr(out=mv[:], in_=stats[:])
            rstd = stat.tile([128, 1], F32, tag="rstd", name="rstd_t")
            nc.scalar.activation(out=rstd[:], in_=mv[:, 1:2],
                                 func=mybir.ActivationFunctionType.Sqrt,
                                 bias=eps_sb[:], scale=1.0)
            nc.vector.reciprocal(out=rstd[:], in_=rstd[:])
            tmp = stat.tile([128, E], F32, tag="ln_tmp", name="ln_tmp")
            nc.vector.tensor_scalar(out=tmp[:], in0=x_ap,
                                    scalar1=mv[:, 0:1], scalar2=rstd[:],
                                    op0=mybir.AluOpType.subtract,
                                    op1=mybir.AluOpType.mult)
            nc.vector.tensor_mul(out=tmp[:], in0=tmp[:], in1=g_bc[:])
            nc.vector.tensor_add(out=out_tile[:], in0=tmp[:], in1=b_bc[:])

        # ---- persistent layer pools (tags reused across layers) ----------
        es_l = es.enter_context(ExitStack())
        wqkp = es_l.enter_context(tc.tile_pool(name="wqkp", bufs=1))
        wvp = es_l.enter_context(tc.tile_pool(name="wvp", bufs=1))
        watpp = es_l.enter_context(tc.tile_pool(name="watpp", bufs=1))
        fcwp = es_l.enter_context(tc.tile_pool(name="fcwp", bufs=1))
        prwp = es_l.enter_context(tc.tile_pool(name="prwp", bufs=1))
        bias_p = es_l.enter_context(tc.tile_pool(name="bias_p", bufs=2))
        hTp = es_l.enter_context(tc.tile_pool(name="hTp", bufs=1))
        hTbp = es_l.enter_context(tc.tile_pool(name="hTbp", bufs=1))
        qkp = es_l.enter_context(tc.tile_pool(name="qkp", bufs=1))
        vp = es_l.enter_context(tc.tile_pool(name="vp", bufs=1))
        ep = es_l.enter_context(tc.tile_pool(name="ep", bufs=2))
        yp = es_l.enter_context(tc.tile_pool(name="yp", bufs=1))
        sm = es_l.enter_context(tc.tile_pool(name="sm", bufs=2))
        mTp = es_l.enter_context(tc.tile_pool(name="mTp", bufs=1))
        yallp = es_l.enter_context(tc.tile_pool(name="yallp", bufs=1))

        # v_sb layout [128, 258]: h0 [v0|1] at 0:65, h1 [1|pad63|v1] at
        # 65:193 (den at psum row 0, y1 at rows 64-127 for packed
        # out-proj), h2 [v2|1] at 193:258. Constant cols written once.
        v_sb = [vp.tile([128, 258], F16, tag=f"v{t}", name=f"v{t}") for t in range(8)]
        for t in range(8):
            nc.vector.tensor_copy(v_sb[t][:, 64:65], ones3[:, 0:1])
            nc.vector.tensor_copy(v_sb[t][:, 65:66], ones3[:, 1:2])
            nc.vector.tensor_copy(v_sb[t][:, 257:258], ones3[:, 2:3])
            nc.vector.memset(v_sb[t][:, 66:129], 0.0)

        def emit_stage(t, g_bc, b_bc, stage_tiles, tin, tag_out, groups):
            """LN(x_sb[t]) -> transpose -> store token-half -> AG trigger."""
            h_t = hpool.tile([128, E], F16, tag="h", name="h")
            layernorm_t(x_sb[t][:], g_bc, b_bc, h_t)
            es_ts = ExitStack()
            psT = es_ts.enter_context(tc.tile_pool(name="psT", bufs=4, space="PSUM"))
            for k in range(6):
                pt = psT.tile([128, 128], F16, tag="tr", name="tr")
                nc.tensor.transpose(pt[:], h_t[:, k * 128:(k + 1) * 128], ident[:])
                dst = stage_tiles[k][:, t * 128:(t + 1) * 128]
                if k % 2 == 0:
                    nc.vector.tensor_copy(dst, pt[:])
                else:
                    nc.scalar.activation(out=dst, in_=pt[:],
                                         func=mybir.ActivationFunctionType.Copy)
            es_ts.close()
            for k in range(6):
                nc.sync.dma_start(out=tin[t][k * 128:(k + 1) * 128, :],
                                  in_=stage_tiles[k][:, t * 128:(t + 1) * 128])
            nc.gpsimd.collective_compute(
                "AllGather", mybir.AluOpType.bypass,
                replica_groups=groups,
                ins=[tin[t].opt()],
                outs=[tag_out[t].opt()],
            )

        def stage_params(i):
            if i < L_RUN:
                g = bcast_row(lnrow, ln1g[i], 128, F16, E, tag="ln1g")
                b = bcast_row(lnrow, ln1b[i], 128, F16, E, tag="ln1b")
                tiles = [hTp.tile([128, TS], F16, tag=f"hT{k}", name=f"hT{k}")
                         for k in range(6)]
                return g, b, tiles, hT_in, hT_ag, g_batch
            g = bcast_row(lnrow, lnfg[0], 128, F16, E, tag="ln1g")
            b = bcast_row(lnrow, lnfb[0], 128, F16, E, tag="ln1b")
            tiles = [hTp.tile([128, TS], F16, tag=f"hT{k}", name=f"xfT{k}")
                     for k in range(6)]
            return g, b, tiles, xf_in, xf_ag, g_all

        # prologue: stage 0 (both token halves, no overlap available)
        sp = stage_params(0)
        for t in range(2):
            emit_stage(t, sp[0], sp[1], sp[2], sp[3], sp[4], sp[5])

        for layer in range(L_RUN):
            # ---- weight loads (gpsimd queue; Tile schedules early) ------
            wqk_sb = [wqkp.tile([128, 512], F16, tag=f"wqk{k}", name=f"wqk{k}") for k in range(6)]
            wv_sb = [wvp.tile([128, 192], F16, tag=f"wv{k}", name=f"wv{k}") for k in range(6)]
            bqk_sb = bias_p.tile([128, 4], F32, tag="bqk", name="bqk")
            nc.sync.dma_start(out=bqk_sb[:], in_=bqk[layer])
            fcb_sb = bias_p.tile([128, 24], F32, tag="fcb", name="fcb")
            nc.sync.dma_start(out=fcb_sb[:], in_=fcb[layer])

            # load hTb 6 x [128, 1024] fp16 (2 token-half DMAs per k-chunk)
            hTb = [hTbp.tile([128, T], F16, tag=f"hTb{k}", name=f"hTb{k}") for k in range(6)]
            for k in range(6):
                for th in range(2):
                    src = hT_ag[th]
                    in_ap = bass.AP(
                        tensor=src.tensor,
                        offset=src[k * 128, 0].offset,
                        ap=[[128, 128], [E * 128, 4], [1, 128]],
                    )
                    eng = (nc.sync, nc.scalar)[(2 * k + th) % 2]
                    eng.dma_start(
                        out=hTb[k][:].rearrange("p (r u tt) -> p r u tt",
                                                r=4, u=2)[:, :, th, :],
                        in_=in_ap)

            if DEBUG_DUMP and layer == 0:
                nc.sync.dma_start(out=dbg[0], in_=hTb[0][:])
                nc.sync.dma_start(out=dbg[11][:, 0:128], in_=tri_sb[:])

            # weight loads on gpsimd AFTER collective triggers (in-order queue)
            watp_sb = [watpp.tile([128, E], F16, tag=f"wa{k}", name=f"wa{k}") for k in range(6)]
            for k in range(6):
                nc.gpsimd.dma_start(out=wqk_sb[k][:], in_=wqk[layer, k * 128:(k + 1) * 128, :])
                nc.gpsimd.dma_start(out=wv_sb[k][:], in_=wv[layer, k * 128:(k + 1) * 128, :])
            for k in range(6):
                nc.gpsimd.dma_start(out=watp_sb[k][:], in_=watp[layer, k * 128:(k + 1) * 128, :])
            fcw_sb = [fcwp.tile([128, 4 * E], F16, tag=f"fcw{k}", name=f"fcw{k}") for k in range(6)]
            for k in range(6):
                nc.gpsimd.dma_start(out=fcw_sb[k][:], in_=fcw[layer, k * 128:(k + 1) * 128, :])
            prw_sb = [prwp.tile([128, E], F16, tag=f"prw{m}", name=f"prw{m}") for m in range(24)]
            for m in range(24):
                nc.gpsimd.dma_start(out=prw_sb[m][:], in_=prw[layer, m * 128:(m + 1) * 128, :])

            # ---- QKV ----------------------------------------------------
            es_a = ExitStack()
            psQK = es_a.enter_context(tc.tile_pool(name="psQK", bufs=3, space="PSUM"))
            psV = es_a.enter_context(tc.tile_pool(name="psV", bufs=2, space="PSUM"))
            # qT2/kT2: rows 0-63 head0, 64-127 head1; q1T/k1T rows 64-127 head2
            qT2 = qkp.tile([128, T], F16, tag="qT2", name="qT2")
            kT2 = qkp.tile([128, T], F16, tag="kT2", name="kT2")
            q1T = qkp.tile([128, T], F16, tag="q1T", name="q1T")
            k1T = qkp.tile([128, T], F16, tag="k1T", name="k1T")
            qk_dsts = [qT2, kT2, q1T, k1T]
            for s in range(4):
                ps = psQK.tile([128, T], F32, tag="qk", name="qk")
                for n in range(2):
                    for k in range(6):
                        nc.tensor.matmul(ps[:, n * 512:(n + 1) * 512],
                                         wqk_sb[k][:, s * 128:(s + 1) * 128],
                                         hTb[k][:, n * 512:(n + 1) * 512],
                                         start=(k == 0), stop=(k == 5))
                if s < 2:
                    nc.vector.tensor_scalar_add(out=qk_dsts[s][:], in0=ps[:],
                                                scalar1=bqk_sb[:, s:s + 1])
                else:
                    nc.vector.tensor_scalar_add(out=qk_dsts[s][64:128, :],
                                                in0=ps[64:128, :],
                                                scalar1=bqk_sb[64:128, s:s + 1])
            for t in range(8):
                ps = psV.tile([128, 192], F32, tag="v", name="v")
                for k in range(6):
                    nc.tensor.matmul(ps[:], hTb[k][:, t * 128:(t + 1) * 128],
                                     wv_sb[k][:], start=(k == 0), stop=(k == 5))
                if t % 2 == 0:
                    nc.vector.tensor_copy(v_sb[t][:, 0:64], ps[:, 0:64])
                    nc.vector.tensor_copy(v_sb[t][:, 129:193], ps[:, 64:128])
                    nc.vector.tensor_copy(v_sb[t][:, 193:257], ps[:, 128:192])
                else:
                    nc.scalar.activation(out=v_sb[t][:, 0:64], in_=ps[:, 0:64],
                                         func=mybir.ActivationFunctionType.Copy)
                    nc.scalar.activation(out=v_sb[t][:, 129:193], in_=ps[:, 64:128],
                                         func=mybir.ActivationFunctionType.Copy)
                    nc.scalar.activation(out=v_sb[t][:, 193:257], in_=ps[:, 128:192],
                                         func=mybir.ActivationFunctionType.Copy)

            if DEBUG_DUMP and layer == 0:
                nc.sync.dma_start(out=dbg[1], in_=qT2[:])
                nc.sync.dma_start(out=dbg[2], in_=kT2[:])
                nc.sync.dma_start(out=dbg[3], in_=k1T[:])
                nc.sync.dma_start(out=dbg[10][:, 0:258], in_=v_sb[0][:])

            # ---- attention: scores + y, head 0/1 row-packed --------------
            es_a.close()
            es_b = ExitStack()
            psS = es_b.enter_context(tc.tile_pool(name="psS", bufs=2, space="PSUM"))
            psY = es_b.enter_context(tc.tile_pool(name="psY", bufs=1, space="PSUM"))
            # h0: y rows 0:64, den 64; h1: den 0, y 64:128; h2: y 0:64, den 64
            yps0 = psY.tile([65, T], F32, tag="y0", name="y0")
            yps1 = psY.tile([128, T], F32, tag="y1", name="y1")
            yps2 = psY.tile([65, T], F32, tag="y2", name="y2")
            heads = [(qT2[0:64, :], kT2[0:64, :], yps0, 0, 65),
                     (qT2[64:128, :], kT2[64:128, :], yps1, 65, 193),
                     (q1T[64:128, :], k1T[64:128, :], yps2, 193, 258)]

            def attn_head(h, j):
                qs = j * 128
                qlen = T - qs
                qT_h, kT_h, yout, v0, v1 = heads[h]
                e_sb = ep.tile([128, T], F16, tag=f"e{h}", name=f"e{h}")
                off = 0
                while off < qlen:
                    cl = min(512, qlen - off)
                    pss = psS.tile([128, 512], F32, tag="s", name="s")
                    nc.tensor.matmul(pss[:, 0:cl], kT_h[:, qs:qs + 128],
                                     qT_h[:, qs + off: qs + off + cl],
                                     start=True, stop=True)
                    nc.scalar.activation(out=e_sb[:, off:off + cl],
                                         in_=pss[:, 0:cl],
                                         func=mybir.ActivationFunctionType.Exp,
                                         scale=SCALE)
                    off += cl
                nc.vector.tensor_mul(out=e_sb[:, 0:128], in0=e_sb[:, 0:128],
                                     in1=tri_sb[:])
                if qs < 512:
                    nc.tensor.matmul(yout[:, qs:512],
                                     v_sb[j][:, v0:v1],
                                     e_sb[:, 0:512 - qs],
                                     start=(j == 0), stop=(j == 3))
                nc.tensor.matmul(yout[:, max(qs, 512):T],
                                 v_sb[j][:, v0:v1],
                                 e_sb[:, max(qs, 512) - qs:qlen],
                                 start=(j == 0), stop=(j == 7))

            def norm_head(h, dst):
                yout = (yps0, yps1, yps2)[h]
                den_row = 0 if h == 1 else 64
                den_sb = sm.tile([1, T], F32, tag="den", name="den")
                nc.vector.tensor_copy(den_sb[:], yout[den_row:den_row + 1, :])
                bcf = sm.tile([128, T], F32, tag="bcf", name="bcf")
                nc.gpsimd.partition_broadcast(bcf[:], den_sb[:])
                lo = 64 if h == 1 else 0
                ysrc = yout[64:128, :] if h == 1 else yout[0:64, :]
                nc.vector.reciprocal_approx_fast(out=bcf[:], in_=bcf[:])
                nc.vector.tensor_mul(out=dst, in0=ysrc, in1=bcf[lo:lo + 64, :])

            # head 2 first: its AG overlaps the packed h0/h1 sweep
            y01T = yp.tile([128, T], F16, tag="y01T", name="y01T")
            y2T = yp.tile([64, T], F16, tag="y2T", name="y2T")
            for j in range(8):
                attn_head(2, j)
            norm_head(2, y2T[:])
            nc.sync.dma_start(out=y2_in[:, :], in_=y2T[:])
            nc.gpsimd.collective_compute(
                "AllGather", mybir.AluOpType.bypass,
                replica_groups=g_batch,
                ins=[y2_in.opt()], outs=[y2_ag.opt()],
            )
            for j in range(8):
                attn_head(0, j)
                attn_head(1, j)
            norm_head(0, y01T[0:64, :])
            norm_head(1, y01T[64:128, :])
            nc.sync.dma_start(out=y01_in[:, :], in_=y01T[:])
            nc.gpsimd.collective_compute(
                "AllGather", mybir.AluOpType.bypass,
                replica_groups=g_batch,
                ins=[y01_in.opt()], outs=[y01_ag.opt()],
            )
            if DEBUG_DUMP and layer == 0:
                nc.sync.dma_start(out=dbg[4], in_=y01T[:])
                nc.sync.dma_start(out=dbg[5][0:64, :], in_=y2T[:])
            es_b.close()

            # own-token slices via rank-dependent column offset:
            # yall k<4 <- y01_ag rank-block k (heads 3k,3k+1);
            # yall 4,5 <- y2_ag (heads 2,5 | 8,11)
            r4 = nc.gpsimd.partition_id() % 4
            yall = [yallp.tile([128, TS], F16, tag=f"ya{k}", name=f"ya{k}") for k in range(6)]
            for k in range(6):
                srcag = y01_ag if k < 4 else y2_ag
                roff = k * 128 if k < 4 else (k - 4) * 128
                in_ap = bass.AP(
                    tensor=srcag.tensor,
                    offset=r4 * TS + srcag[roff, 0].offset,
                    ap=[[T, 128], [1, TS]],
                    dep_tracking_offset=srcag[roff, 0].offset,
                )
                nc.gpsimd.dma_start(out=yall[k][:], in_=in_ap)

            # ---- out-proj (own 256 tokens, full 768 contraction) ---------
            es_c = ExitStack()
            psO = es_c.enter_context(tc.tile_pool(name="psO", bufs=2, space="PSUM"))
            atpb_bc = bcast_row(lnrow, atpb[layer], 128, F16, E, tag="atpb")
            for t in range(2):
                po = psO.tile([128, E], F32, tag="o", name="o")
                for n0, n1 in ((0, 512), (512, 768)):
                    for k in range(6):
                        nc.tensor.matmul(po[:, n0:n1],
                                         yall[k][:, t * 128:(t + 1) * 128],
                                         watp_sb[k][:, n0:n1],
                                         start=(k == 0), stop=(k == 5))
                nc.vector.tensor_add(out=x_sb[t][:], in0=x_sb[t][:], in1=po[:])
                nc.vector.tensor_add(out=x_sb[t][:], in0=x_sb[t][:], in1=atpb_bc[:])
            if DEBUG_DUMP and layer == 0:
                for t in range(2):
                    xc = hpool.tile([128, E], F16, tag="ao", name="xc")
                    nc.vector.tensor_copy(xc[:], x_sb[t][:])
                    nc.sync.dma_start(out=dbg[8 + t][:, 0:E], in_=xc[:])
            es_c.close()

            # ---- LN2 + transpose -> h2T ---------------------------------
            ln2g_bc = bcast_row(lnrow, ln2g[layer], 128, F16, E, tag="ln2g")
            ln2b_bc = bcast_row(lnrow, ln2b[layer], 128, F16, E, tag="ln2b")
            es_t2 = ExitStack()
            psT2 = es_t2.enter_context(tc.tile_pool(name="psT2", bufs=4, space="PSUM"))
            h2T = [hTp.tile([128, TS], F16, tag=f"h2T{k}", name=f"h2T{k}") for k in range(6)]
            for t in range(2):
                h_t = hpool.tile([128, E], F16, tag="h", name="h")
                layernorm_t(x_sb[t][:], ln2g_bc, ln2b_bc, h_t)
                for k in range(6):
                    pt = psT2.tile([128, 128], F16, tag="tr2", name="tr2")
                    nc.tensor.transpose(pt[:], h_t[:, k * 128:(k + 1) * 128], ident[:])
                    dst = h2T[k][:, t * 128:(t + 1) * 128]
                    if k % 2 == 0:
                        nc.vector.tensor_copy(dst, pt[:])
                    else:
                        nc.scalar.activation(out=dst, in_=pt[:],
                                             func=mybir.ActivationFunctionType.Copy)
            es_t2.close()

            # ---- MLP fc: mT[m] = gelu(fcw[:,m].T @ h2T + fcb[m]) ---------
            es_fc = ExitStack()
            psM = es_fc.enter_context(tc.tile_pool(name="psM", bufs=4, space="PSUM"))
            mT = [mTp.tile([128, TS], F16, tag=f"mT{m}", name=f"mT{m}") for m in range(24)]
            if SIM_GELU:
                fcb17 = bias_p.tile([128, 24], F32, tag="fcb17", name="fcb17")
                nc.vector.tensor_scalar(out=fcb17[:], in0=fcb_sb[:], scalar1=1.702,
                                        scalar2=None,
                                        op0=mybir.AluOpType.mult)
            for m in range(24):
                ps = psM.tile([128, TS], F32, tag="m", name="m")
                for k in range(6):
                    nc.tensor.matmul(ps[:], fcw_sb[k][:, m * 128:(m + 1) * 128],
                                     h2T[k][:], start=(k == 0), stop=(k == 5))
                if SIM_GELU:
                    sgm = mTp.tile([128, TS], F32, tag="sgm", name="sgm")
                    nc.scalar.activation(out=sgm[:], in_=ps[:],
                                         func=mybir.ActivationFunctionType.Sigmoid,
                                         scale=1.702, bias=fcb17[:, m:m + 1])
                    nc.vector.tensor_scalar_add(out=mT[m][:], in0=ps[:],
                                                scalar1=fcb_sb[:, m:m + 1])
                    nc.vector.tensor_mul(out=mT[m][:], in0=mT[m][:], in1=sgm[:])
                else:
                    nc.scalar.activation(out=mT[m][:], in_=ps[:],
                                         func=mybir.ActivationFunctionType.Gelu_apprx_tanh,
                                         bias=fcb_sb[:, m:m + 1])

            es_fc.close()
            # ---- MLP pr + residual, t-split; emit next stage per half ----
            es_d = ExitStack()
            psP = es_d.enter_context(tc.tile_pool(name="psP", bufs=1, space="PSUM"))
            prb_bc = bcast_row(lnrow, prb[layer], 128, F16, E, tag="prb")
            spn = stage_params(layer + 1)
            for t in range(2):
                ps2 = psP.tile([128, E], F32, tag=f"p{t}", name=f"p{t}")
                for m in range(24):
                    for n0, n1 in ((0, 512), (512, 768)):
                        nc.tensor.matmul(ps2[:, n0:n1],
                                         mT[m][:, t * 128:(t + 1) * 128],
                                         prw_sb[m][:, n0:n1],
                                         start=(m == 0), stop=(m == 23))
                nc.vector.tensor_add(out=x_sb[t][:], in0=x_sb[t][:], in1=ps2[:])
                nc.vector.tensor_add(out=x_sb[t][:], in0=x_sb[t][:], in1=prb_bc[:])
                emit_stage(t, spn[0], spn[1], spn[2], spn[3], spn[4], spn[5])
            es_d.close()

        # ---- lm_head (xf stage emitted by last layer / prologue) --------
        es_l.close()
        es_h = es.enter_context(ExitStack())
        xfp = es_h.enter_context(tc.tile_pool(name="xfp", bufs=1))
        wtep = es_h.enter_context(tc.tile_pool(name="wtep", bufs=2))
        lop = es_h.enter_context(tc.tile_pool(name="lop", bufs=4))
        psL = es_h.enter_context(tc.tile_pool(name="psL", bufs=2, space="PSUM"))

        # xfT_full 6 x [128, 2048] fp16 (3D-AP load across 8 ranks)
        xf_full = [xfp.tile([128, B * T], F16, tag=f"xf{k}", name=f"xf{k}") for k in range(6)]
        for k in range(6):
            for th in range(2):
                src_t = xf_ag[th]
                in_ap = bass.AP(
                    tensor=src_t.tensor,
                    offset=src_t[k * 128, 0].offset,
                    ap=[[128, 128], [E * 128, NC], [1, 128]],
                )
                eng = (nc.sync, nc.scalar)[(2 * k + th) % 2]
                eng.dma_start(
                    out=xf_full[k][:].rearrange("p (r u tt) -> p r u tt",
                                                r=NC, u=2)[:, :, th, :],
                    in_=in_ap)

        nch = (VS + 511) // 512
        for n in range(nch):
            n0 = n * 512
            nw = min(512, VS - n0)
            wte_sb = [wtep.tile([128, 512], F16, tag=f"wte{k}", name=f"wte{k}") for k in range(6)]
            for k in range(6):
                nc.gpsimd.dma_start(out=wte_sb[k][:, 0:nw],
                                    in_=wteT[n, k, :, 0:nw])
            for q in range(4):
                pss = [psL.tile([128, 512], F32, tag=f"l{t}", name=f"l{t}") for t in range(4)]
                for k in range(6):
                    for t in range(4):
                        nc.tensor.matmul(pss[t][:, 0:nw],
                                         xf_full[k][:, (q * 4 + t) * 128:(q * 4 + t + 1) * 128],
                                         wte_sb[k][:, 0:nw],
                                         start=(k == 0), stop=(k == 5))
                for t in range(4):
                    lo = lop.tile([128, 512], F16, tag="lo", name="lo")
                    if t % 2 == 0:
                        nc.vector.tensor_copy(lo[:, 0:nw], pss[t][:, 0:nw])
                    else:
                        nc.scalar.activation(out=lo[:, 0:nw], in_=pss[t][:, 0:nw],
                                             func=mybir.ActivationFunctionType.Copy)
                    eng = nc.sync if t % 2 == 0 else nc.scalar
                    row0 = (n * 16 + q * 4 + t) * 128
                    eng.dma_start(out=logits[row0:row0 + 128, 0:nw],
                                  in_=lo[:, 0:nw])

    nc.compile()
    return nc


def _block_wte(wt, nch, vs_pad):
    # [768, VS] -> [nch, 6, 128, 512] fp16 blocked
    pad = np.zeros((E, vs_pad - wt.shape[1]), np.float32)
    wtp = np.concatenate([wt, pad], axis=1)
    return np.ascontiguousarray(
        wtp.reshape(6, 128, nch, 512).transpose(2, 0, 1, 3).astype(np.float16))


def _prep_inputs(idx, wte, wpe, ln1_w, ln1_b, attn_w, attn_b, atp_w, atp_b,
                 ln2_w, ln2_b, fc_w, fc_b, pr_w, pr_b, lnf_w, lnf_b):
    idx = np.asarray(idx)
    f = lambda a: np.ascontiguousarray(np.asarray(a), dtype=np.float32)
    h = lambda a: np.ascontiguousarray(np.asarray(a), dtype=np.float16)
    wte, wpe = f(wte), f(wpe)
    x0 = wte[idx.reshape(-1)] + np.tile(wpe[:T], (B, 1))  # [2048, 768]
    wte_pad = np.zeros((VPAD, E), np.float32)
    wte_pad[:V] = wte
    wteT_full = np.ascontiguousarray(wte_pad.T)  # [768, VPAD]
    nch = (VS + 511) // 512
    vs_pad = nch * 512

    attn_w, attn_b = f(attn_w), f(attn_b)
    atp_w, atp_b = f(atp_w), f(atp_b)
    fc_w, fc_b, pr_w, pr_b = f(fc_w), f(fc_b), f(pr_w), f(pr_b)

    # fold v-bias through atp: y_true = y/den + bv  ->  + bv @ atp_w
    bv_full = attn_b[:, 2 * E:]                       # [L, 768]
    atpb_eff = atp_b + np.einsum('le,leo->lo', bv_full, atp_w)

    in_maps = []
    for c in range(NC):
        hs = 3 * (c % 4)
        q = [attn_w[:, :, (hs + hh) * HD:(hs + hh + 1) * HD] for hh in range(3)]
        k = [attn_w[:, :, E + (hs + hh) * HD:E + (hs + hh + 1) * HD] for hh in range(3)]
        v = [attn_w[:, :, 2 * E + (hs + hh) * HD:2 * E + (hs + hh + 1) * HD] for hh in range(3)]
        pad = np.zeros((L, E, HD), np.float32)
        # cols: [q0|q1, k0|k1, pad|q2, pad|k2]
        wqk_c = np.concatenate([q[0], q[1], k[0], k[1], pad, q[2], pad, k[2]], axis=2)
        qb = [attn_b[:, (hs + hh) * HD:(hs + hh + 1) * HD] for hh in range(3)]
        kb = [attn_b[:, E + (hs + hh) * HD:E + (hs + hh + 1) * HD] for hh in range(3)]
        zb = np.zeros((L, HD), np.float32)
        bqk_c = np.stack([
            np.concatenate([qb[0], qb[1]], axis=1),
            np.concatenate([kb[0], kb[1]], axis=1),
            np.concatenate([zb, qb[2]], axis=1),
            np.concatenate([zb, kb[2]], axis=1),
        ], axis=2)  # [L, 128, 4]
        wv_c = np.concatenate(v, axis=2)
        in_maps.append({
            "x0s": np.ascontiguousarray(x0[c * TS:(c + 1) * TS]),
            "wqk": h(wqk_c), "bqk": np.ascontiguousarray(bqk_c),
            "wv": h(wv_c),
            "watp": h(atp_w.reshape(L, H, HD, E)[
                :, [g for r in range(4) for g in (3 * r, 3 * r + 1)] +
                [3 * r + 2 for r in range(4)]].reshape(L, E, E)),
            "atpb": np.ascontiguousarray(atpb_eff),
            "fcw": h(fc_w), "fcb": np.ascontiguousarray(
                fc_b.reshape(L, 24, 128).transpose(0, 2, 1)),
            "prw": h(pr_w), "prb": pr_b,
            "ln1g": f(ln1_w), "ln1b": f(ln1_b),
            "ln2g": f(ln2_w), "ln2b": f(ln2_b),
            "lnfg": f(lnf_w).reshape(1, E), "lnfb": f(lnf_b).reshape(1, E),
            "wteT": _block_wte(wteT_full[:, c * VS:(c + 1) * VS], nch, vs_pad),
        })
    return in_maps


def kernel(trace=False, **inputs):
    if "nc" not in _CACHE:
        _CACHE["nc"] = _build_program()
    nc = _CACHE["nc"]
    in_maps = _prep_inputs(**inputs)
    res = run_bass_kernel_spmd(nc, in_maps, core_ids=list(range(NC)), trace=trace)
    _CACHE["last_result"] = res
    nch = (VS + 511) // 512
    full = np.empty((B * T, V), np.float32)
    for c in range(NC):
        blk = res.results[c]["logits"].reshape(nch, 16 * 128, 512)
        for n in range(nch):
            n0 = c * VS + n * 512
            nw = min(512, VS - n * 512)
            lo = blk[n][:, :nw]
            v0 = min(n0, V)
            v1 = min(n0 + nw, V)
            if v1 > v0:
                full[:, v0:v1] = lo[:, :v1 - v0]
    return full.reshape(B, T, V)
